# revision 12
# baseline (speedup 1.0000x reference)
"""Trainium2 Bass kernel for a Switch-style top-1 MoE block (BantamMoEBlock).

Strategy (8 NeuronCores, one full TRN2 chip):
  Launch 1 (router, data-parallel): core c takes token shard c (2048 tokens),
    computes logits = x @ router_w + router_b on the tensor engine (fp32),
    softmax stats + top-1 expert id per token, and the partial sums needed
    for the aux loss (sum of probs per expert, sum of logsumexp^2).
  Host: merges the 8 shards' routing decisions, applies the capacity
    truncation (first-come, cap = 2560), and builds per-expert gather lists.
  Launch 2 (experts, expert-parallel): core e owns expert e. It gathers its
    tokens' rows from a local replica of x via indirect DMA, transposes them
    on the tensor engine, and runs the SwiGLU expert in bf16
    (y = (silu(x@w1+b1) * (x@w2+b2)) @ w3 + b3) with fp32 PSUM accumulation.
  Host: scatters the per-expert outputs back to token order (rows are
    disjoint for top-1 routing) and assembles the scalar aux loss from the
    device partial sums.
"""

import sys

sys.path.insert(0, "/opt/trn_rl_repo")

import numpy as np

import concourse.bass as bass
import concourse.bacc as bacc
import concourse.bass_isa as bass_isa
import concourse.mybir as mybir
import concourse.tile as tile
from concourse.masks import make_identity
from concourse.bass_utils import run_bass_kernel_spmd

P = 128
B, T, H, F, E = 8, 2048, 1024, 2048, 8
S = B * T
CAP = int(np.ceil(1.25 * S / float(E)))  # 2560
NROWS0 = 2304  # 18 groups of 128; covers the worst expert load with slack
AUX_W, Z_W = 0.01, 0.001

FP32 = mybir.dt.float32
BF16 = mybir.dt.bfloat16
I32 = mybir.dt.int32
U32 = mybir.dt.uint32
AF = mybir.ActivationFunctionType
ALU = mybir.AluOpType

N_CORES = 8
TSHARD = S // N_CORES  # 2048 tokens per core in launch 1
KT_H = H // P  # 8 k-tiles over H
KT_F = F // P  # 16 k-tiles over F

# CoreSim does not implement the Silu activation table; set True to build the
# expert kernel with an equivalent sigmoid(x)*x decomposition (sim testing).
SILU_DECOMPOSE = False


def build_router_nc():
    nc = bacc.Bacc("TRN2", target_bir_lowering=False)
    xs = nc.dram_tensor("xs", [TSHARD, H], FP32, kind="ExternalInput")
    rw = nc.dram_tensor("rw", [P, KT_H * E], FP32, kind="ExternalInput")  # [p, k*8+e]
    rb = nc.dram_tensor("rb", [E], FP32, kind="ExternalInput")
    idx_out = nc.dram_tensor("idx_out", [P, TSHARD // P], U32, kind="ExternalOutput")
    ps_out = nc.dram_tensor("ps_out", [1, E], FP32, kind="ExternalOutput")
    z2_out = nc.dram_tensor("z2_out", [1, 1], FP32, kind="ExternalOutput")

    NT = TSHARD // P  # 16 token tiles

    with tile.TileContext(nc) as tc:
        with (
            tc.tile_pool(name="const", bufs=1) as cpool,
            tc.tile_pool(name="work", bufs=3) as wpool,
            tc.tile_pool(name="acc", bufs=1) as apool,
            tc.tile_pool(name="psum", bufs=2, space="PSUM") as pp,
            tc.tile_pool(name="psum_lg", bufs=2, space="PSUM") as plg,
            tc.tile_pool(name="psum_lg2", bufs=2, space="PSUM") as plg2,
        ):
            ident = cpool.tile([P, P], FP32, tag="ident")
            make_identity(nc, ident[:])
            # first token tile's DMA issued before anything else on the queue
            xs0_sb = wpool.tile([P, H], FP32, tag="xs")
            nc.sync.dma_start(xs0_sb[:], xs[0:P, :])
            rw_sb = cpool.tile([P, KT_H, E], FP32, tag="rw")
            nc.sync.dma_start(rw_sb[:], rw[:].rearrange("p (k e) -> p k e", e=E))
            ones_sb = cpool.tile([1, P], FP32, tag="ones")
            nc.vector.memset(ones_sb[:], 1.0)
            rb_sb = cpool.tile([1, E], FP32, tag="rb")
            nc.sync.dma_start(rb_sb[:], rb[None, :])
            iota8 = cpool.tile([P, E], FP32, tag="iota8")
            iota8i = cpool.tile([P, E], I32, tag="iota8i")
            nc.gpsimd.iota(iota8i[:], [[1, E]], channel_multiplier=0)
            nc.vector.tensor_copy(iota8[:], iota8i[:])

            lgall = apool.tile([P, NT, E], FP32, tag="lgall")

            for t in range(NT):
                if t == 0:
                    xs_sb = xs0_sb
                else:
                    xs_sb = wpool.tile([P, H], FP32, tag="xs")
                    nc.sync.dma_start(xs_sb[:], xs[t * P : (t + 1) * P, :])
                xt_sb = wpool.tile([P, KT_H, P], FP32, tag="xt")
                for k in range(KT_H):
                    tr_ps = pp.tile([P, P], FP32, tag="tr")
                    nc.tensor.transpose(
                        tr_ps[:], xs_sb[:, k * P : (k + 1) * P], ident[:]
                    )
                    nc.vector.tensor_copy(xt_sb[:, k, :], tr_ps[:])
                # logits.T [E, P] with router weights as the stationary operand
                lgT_ps = plg.tile([E, P], FP32, tag="lgT")
                for k in range(KT_H):
                    nc.tensor.matmul(
                        lgT_ps[:],
                        lhsT=rw_sb[:, k, :],
                        rhs=xt_sb[:, k, :],
                        start=(k == 0),
                        stop=False,
                    )
                # += rb.T @ ones : adds router_b[e] to every token's logits
                nc.tensor.matmul(
                    lgT_ps[:],
                    lhsT=rb_sb[0:1, :],
                    rhs=ones_sb[0:1, :],
                    start=False,
                    stop=True,
                )
                lgT_sb = wpool.tile([E, P], FP32, tag="lgT_sb")
                nc.vector.tensor_copy(lgT_sb[:], lgT_ps[:])
                lg_ps2 = plg2.tile([P, E], FP32, tag="lg2")
                nc.tensor.transpose(lg_ps2[:], lgT_sb[:], ident[0:E, 0:E])
                nc.vector.tensor_copy(lgall[:, t, :], lg_ps2[:])

            # batched softmax / logsumexp / argmax stats over all NT tiles
            m16 = apool.tile([P, NT], FP32, tag="m16")
            nc.vector.tensor_reduce(
                m16[:], lgall[:], axis=mybir.AxisListType.X, op=ALU.max
            )
            m16b = m16[:, :, None].to_broadcast([P, NT, E])
            lgc = apool.tile([P, NT, E], FP32, tag="lgc")
            nc.vector.tensor_tensor(lgc[:], lgall[:], m16b, op=ALU.subtract)
            pex = apool.tile([P, NT, E], FP32, tag="pex")
            nc.scalar.activation(
                pex[:].rearrange("p t e -> p (t e)"),
                lgc[:].rearrange("p t e -> p (t e)"),
                AF.Exp,
            )
            se16 = apool.tile([P, NT], FP32, tag="se16")
            nc.vector.tensor_reduce(
                se16[:], pex[:], axis=mybir.AxisListType.X, op=ALU.add
            )
            rse16 = apool.tile([P, NT], FP32, tag="rse16")
            nc.vector.reciprocal(rse16[:], se16[:])
            pn = apool.tile([P, NT, E], FP32, tag="pn")
            nc.vector.tensor_tensor(
                pn[:], pex[:], rse16[:, :, None].to_broadcast([P, NT, E]), op=ALU.mult
            )
            psum8 = apool.tile([P, E], FP32, tag="psum8")
            nc.vector.tensor_reduce(
                psum8[:], pn[:].rearrange("p t e -> p e t"),
                axis=mybir.AxisListType.X, op=ALU.add,
            )
            # argmax per (token): sum(e * (lg == max))
            eq = apool.tile([P, NT, E], FP32, tag="eq")
            nc.vector.tensor_tensor(eq[:], lgall[:], m16b, op=ALU.is_equal)
            nc.vector.tensor_tensor(
                eq[:], eq[:], iota8[:, None, :].to_broadcast([P, NT, E]), op=ALU.mult
            )
            idxf = apool.tile([P, NT], FP32, tag="idxf")
            nc.vector.tensor_reduce(
                idxf[:], eq[:], axis=mybir.AxisListType.X, op=ALU.add
            )
            idxall = apool.tile([P, NT], U32, tag="idxall")
            nc.vector.tensor_copy(idxall[:], idxf[:])

            lnse = apool.tile([P, NT], FP32, tag="lnse")
            nc.scalar.activation(lnse[:], se16[:], AF.Ln)
            zall = apool.tile([P, NT], FP32, tag="zall")
            nc.vector.tensor_tensor(zall[:], lnse[:], m16[:], op=ALU.add)
            z2p = apool.tile([P, 1], FP32, tag="z2p")
            z2sq = apool.tile([P, NT], FP32, tag="z2sq")
            nc.scalar.activation(z2sq[:], zall[:], AF.Square, accum_out=z2p[:])

            z2r = apool.tile([P, 1], FP32, tag="z2r")
            nc.gpsimd.partition_all_reduce(
                z2r[:], z2p[:], channels=P, reduce_op=bass_isa.ReduceOp.add
            )
            pr = apool.tile([P, E], FP32, tag="pr")
            nc.gpsimd.partition_all_reduce(
                pr[:], psum8[:], channels=P, reduce_op=bass_isa.ReduceOp.add
            )
            nc.sync.dma_start(idx_out[:], idxall[:])
            nc.sync.dma_start(ps_out[:], pr[0:1, :])
            nc.sync.dma_start(z2_out[:], z2r[0:1, :])
    nc.finalize()
    return nc


def _chunks(nrows):
    out = []
    pos = 0
    while pos < nrows:
        c = min(1024, nrows - pos)
        out.append((pos, c))
        pos += c
    return out


def build_expert_nc(nrows):
    assert nrows % P == 0
    ng = nrows // P
    nc = bacc.Bacc("TRN2", target_bir_lowering=False)
    xf = nc.dram_tensor("xf", [S, H], FP32, kind="ExternalInput")
    idx = nc.dram_tensor("idx", [P, ng], I32, kind="ExternalInput")
    w1 = nc.dram_tensor("w1", [H, F], FP32, kind="ExternalInput")
    w2 = nc.dram_tensor("w2", [H, F], FP32, kind="ExternalInput")
    w3 = nc.dram_tensor("w3", [F, H], FP32, kind="ExternalInput")
    b1 = nc.dram_tensor("b1", [P, KT_F], FP32, kind="ExternalInput")  # b1[f] at [f%128, f//128]
    b2 = nc.dram_tensor("b2", [P, KT_F], FP32, kind="ExternalInput")
    b3 = nc.dram_tensor("b3", [1, H], FP32, kind="ExternalInput")
    y_out = nc.dram_tensor("y_out", [nrows, H], FP32, kind="ExternalOutput")

    with tile.TileContext(nc) as tc:
        with (
            tc.tile_pool(name="const", bufs=1) as cpool,
            tc.tile_pool(name="wres", bufs=1) as wres,
            tc.tile_pool(name="wstage", bufs=2) as wstage,
            tc.tile_pool(name="xg", bufs=2) as xg,
            tc.tile_pool(name="xgb", bufs=2) as xgb,
            tc.tile_pool(name="xt", bufs=1) as xtp,
            tc.tile_pool(name="gt", bufs=1) as gtp,
            tc.tile_pool(name="ysb", bufs=2) as ysb,
            tc.tile_pool(name="ew", bufs=2) as ew,
            tc.tile_pool(name="ps_tr", bufs=2, space="PSUM") as ps_tr,
            tc.tile_pool(name="ps_h1", bufs=1, space="PSUM") as ps_h1,
            tc.tile_pool(name="ps_h2", bufs=1, space="PSUM") as ps_h2,
            tc.tile_pool(name="ps_y", bufs=2, space="PSUM") as ps_y,
        ):
            ident_bf = cpool.tile([P, P], BF16, tag="identbf")
            make_identity(nc, ident_bf[:])
            idx_sb = cpool.tile([P, ng], I32, tag="idx")
            nc.sync.dma_start(idx_sb[:], idx[:])
            b1_sb = cpool.tile([P, KT_F], FP32, tag="b1")
            nc.sync.dma_start(b1_sb[:], b1[:])
            b2_sb = cpool.tile([P, KT_F], FP32, tag="b2")
            nc.sync.dma_start(b2_sb[:], b2[:])
            b3_row = cpool.tile([1, H], FP32, tag="b3row")
            nc.sync.dma_start(b3_row[:], b3[:])
            b3_bc = cpool.tile([P, H], FP32, tag="b3bc")
            nc.gpsimd.partition_broadcast(b3_bc[:], b3_row[0:1, :])

            def emit_chunk_load(c0, clen):
                ngr = clen // P
                g0 = c0 // P
                xt_sb = xtp.tile([P, KT_H, 1024], BF16, tag="xt")
                for g in range(ngr):
                    xe_sb = xg.tile([P, H], FP32, tag="xe")
                    nc.gpsimd.indirect_dma_start(
                        out=xe_sb[:],
                        out_offset=None,
                        in_=xf[:],
                        in_offset=bass.IndirectOffsetOnAxis(
                            ap=idx_sb[:, g0 + g : g0 + g + 1], axis=0
                        ),
                    )
                    xe_bf = xgb.tile([P, H], BF16, tag="xeb")
                    nc.vector.tensor_copy(xe_bf[:], xe_sb[:])
                    for k in range(KT_H):
                        tr_ps = ps_tr.tile([P, P], BF16, tag="tr")
                        nc.tensor.transpose(
                            tr_ps[:], xe_bf[:, k * P : (k + 1) * P], ident_bf[:]
                        )
                        nc.vector.tensor_copy(
                            xt_sb[:, k, g * P : (g + 1) * P], tr_ps[:]
                        )
                return xt_sb

            chunks = _chunks(nrows)
            # chunk 0 token loads first so the tensor engine warms up early
            xt_first = emit_chunk_load(*chunks[0])

            # resident bf16 weights, staged in consumption order:
            # (w1 c, w2 c) slices for c = 0..3, then w3 (used by the y matmuls)
            w1_bf = wres.tile([P, KT_H, F], BF16, tag="w1")
            w2_bf = wres.tile([P, KT_H, F], BF16, tag="w2")
            w3_bf = wres.tile([P, KT_F, H], BF16, tag="w3")

            def stage(wdram, wbf, k, c):
                st = wstage.tile([P, 512], FP32, tag="wst")
                nc.sync.dma_start(
                    st[:], wdram[k * P : (k + 1) * P, c * 512 : (c + 1) * 512]
                )
                nc.vector.tensor_copy(wbf[:, k, c * 512 : (c + 1) * 512], st[:])

            for c in range(F // 512):
                for k in range(KT_H):
                    stage(w1, w1_bf, k, c)
                for k in range(KT_H):
                    stage(w2, w2_bf, k, c)
            for c in range(H // 512):
                for k in range(KT_F):
                    stage(w3, w3_bf, k, c)

            for ci, (c0, clen) in enumerate(chunks):
                ngr = clen // P
                xt_sb = xt_first if ci == 0 else emit_chunk_load(c0, clen)

                gt_sb = gtp.tile([P, KT_F, 1024], BF16, tag="gt")
                halves = [(h0, min(512, clen - h0)) for h0 in range(0, clen, 512)]
                for ft in range(KT_F):
                    h1_ps = ps_h1.tile([P, 1024], FP32, tag="h1")
                    h2_ps = ps_h2.tile([P, 1024], FP32, tag="h2")
                    for hps, wbf in ((h1_ps, w1_bf), (h2_ps, w2_bf)):
                        for h0, hlen in halves:
                            for k in range(KT_H):
                                nc.tensor.matmul(
                                    hps[:, h0 : h0 + hlen],
                                    lhsT=wbf[:, k, ft * P : (ft + 1) * P],
                                    rhs=xt_sb[:, k, h0 : h0 + hlen],
                                    start=(k == 0),
                                    stop=(k == KT_H - 1),
                                )
                    if SILU_DECOMPOSE:
                        sg = ew.tile([P, 1024], FP32, tag="sg")
                        nc.scalar.activation(
                            sg[:, :clen], h1_ps[:, :clen], AF.Sigmoid,
                            bias=b1_sb[:, ft : ft + 1], scale=1.0,
                        )
                        h1b = ew.tile([P, 1024], FP32, tag="h1b")
                        nc.scalar.activation(
                            h1b[:, :clen], h1_ps[:, :clen], AF.Identity,
                            bias=b1_sb[:, ft : ft + 1], scale=1.0,
                        )
                        s1 = ew.tile([P, 1024], BF16, tag="s1")
                        nc.vector.tensor_tensor(
                            s1[:, :clen], sg[:, :clen], h1b[:, :clen], op=ALU.mult
                        )
                    else:
                        s1 = ew.tile([P, 1024], BF16, tag="s1")
                        nc.scalar.activation(
                            s1[:, :clen], h1_ps[:, :clen], AF.Silu,
                            bias=b1_sb[:, ft : ft + 1], scale=1.0,
                        )
                    h2b = ew.tile([P, 1024], BF16, tag="h2b")
                    nc.scalar.activation(
                        h2b[:, :clen], h2_ps[:, :clen], AF.Identity,
                        bias=b2_sb[:, ft : ft + 1], scale=1.0,
                    )
                    nc.vector.tensor_tensor(
                        gt_sb[:, ft, :clen], s1[:, :clen], h2b[:, :clen], op=ALU.mult
                    )

                for t in range(ngr):
                    y_sb = ysb.tile([P, H], FP32, tag="y")
                    for hc in range(H // 512):
                        y_ps = ps_y.tile([P, 512], FP32, tag="y")
                        for kf in range(KT_F):
                            nc.tensor.matmul(
                                y_ps[:],
                                lhsT=gt_sb[:, kf, t * P : (t + 1) * P],
                                rhs=w3_bf[:, kf, hc * 512 : (hc + 1) * 512],
                                start=(kf == 0),
                                stop=(kf == KT_F - 1),
                            )
                        nc.vector.tensor_tensor(
                            y_sb[:, hc * 512 : (hc + 1) * 512],
                            y_ps[:],
                            b3_bc[:, hc * 512 : (hc + 1) * 512],
                            op=ALU.add,
                        )
                    nc.sync.dma_start(
                        y_out[c0 + t * P : c0 + (t + 1) * P, :], y_sb[:]
                    )
    nc.finalize()
    return nc


_NC_CACHE = {}


def _get_nc(kind, *args):
    key = (kind,) + args
    if key not in _NC_CACHE:
        if kind == "router":
            _NC_CACHE[key] = build_router_nc()
        else:
            _NC_CACHE[key] = build_expert_nc(*args)
    return _NC_CACHE[key]


def _run(nc, in_maps, **kw):
    return run_bass_kernel_spmd(nc, in_maps, core_ids=list(range(N_CORES)), **kw)


def kernel(x, router_w, router_b, w1, b1, w2, b2, w3, b3, _trace=None):
    x = np.ascontiguousarray(np.asarray(x, np.float32))
    router_w = np.ascontiguousarray(np.asarray(router_w, np.float32))
    router_b = np.ascontiguousarray(np.asarray(router_b, np.float32))
    w1 = np.ascontiguousarray(np.asarray(w1, np.float32))
    b1 = np.ascontiguousarray(np.asarray(b1, np.float32))
    w2 = np.ascontiguousarray(np.asarray(w2, np.float32))
    b2 = np.ascontiguousarray(np.asarray(b2, np.float32))
    w3 = np.ascontiguousarray(np.asarray(w3, np.float32))
    b3 = np.ascontiguousarray(np.asarray(b3, np.float32))

    X = x.reshape(S, H)
    # router_w rearranged so SBUF partition p holds rw[k*128+p, e] at [p, k*E+e]
    rw_r = np.ascontiguousarray(
        router_w.reshape(KT_H, P, E).transpose(1, 0, 2).reshape(P, KT_H * E)
    )

    nc1 = _get_nc("router")
    in_maps1 = [
        {
            "xs": X[c * TSHARD : (c + 1) * TSHARD],
            "rw": rw_r,
            "rb": router_b,
        }
        for c in range(N_CORES)
    ]
    res1 = _run(nc1, in_maps1, **({"trace": True, "tmpdir": _trace + "/l1"} if _trace else {}))

    experts = np.empty(S, np.int64)
    ps_total = np.zeros(E, np.float64)
    z2_total = 0.0
    for c in range(N_CORES):
        r = res1.results[c]
        experts[c * TSHARD : (c + 1) * TSHARD] = (
            r["idx_out"].astype(np.int64).T.reshape(TSHARD)
        )
        ps_total += r["ps_out"][0].astype(np.float64)
        z2_total += float(r["z2_out"][0, 0])
    l1_ns = res1.exec_time_ns

    # capacity truncation (drop_policy='first'), gather lists
    kept_lists = []
    load = np.zeros(E, np.float64)
    for e in range(E):
        toks = np.nonzero(experts == e)[0][:CAP]
        kept_lists.append(toks)
        load[e] = len(toks)
    max_kept = max(len(t) for t in kept_lists)
    nrows = NROWS0 if max_kept <= NROWS0 else CAP
    ng = nrows // P

    nc2 = _get_nc("expert", nrows)
    in_maps2 = []
    for e in range(E):
        toks = kept_lists[e]
        padded = np.zeros(nrows, np.int32)
        padded[: len(toks)] = toks
        idx_arr = np.ascontiguousarray(padded.reshape(ng, P).T)
        b1r = np.ascontiguousarray(b1[e].reshape(KT_F, P).T)
        b2r = np.ascontiguousarray(b2[e].reshape(KT_F, P).T)
        in_maps2.append(
            {
                "xf": X,
                "idx": idx_arr,
                "w1": w1[e],
                "w2": w2[e],
                "w3": w3[e],
                "b1": b1r,
                "b2": b2r,
                "b3": b3[e][None, :],
            }
        )
    res2 = _run(nc2, in_maps2, **({"trace": True, "tmpdir": _trace + "/l2"} if _trace else {}))

    out = np.zeros((S, H), np.float32)
    for e in range(E):
        toks = kept_lists[e]
        out[toks] = res2.results[e]["y_out"][: len(toks)]
    l2_ns = res2.exec_time_ns

    importance = ps_total / float(S)
    aux = float((importance * (load / float(S))).sum() * E * AUX_W)
    aux += z2_total / float(S) * Z_W
    aux = np.float32(aux)

    if _trace:
        kernel.last_exec_ns = (l1_ns, l2_ns)
    return out.reshape(B, T, H), aux


# revision 13
# speedup vs baseline: 1.0293x; 1.0293x over previous
"""Trainium2 Bass kernel for a Switch-style top-1 MoE block (BantamMoEBlock).

Strategy (8 NeuronCores, one full TRN2 chip):
  Launch 1 (router, data-parallel): core c takes token shard c (2048 tokens),
    computes logits = x @ router_w + router_b on the tensor engine (fp32),
    softmax stats + top-1 expert id per token, and the partial sums needed
    for the aux loss (sum of probs per expert, sum of logsumexp^2).
  Host: merges the 8 shards' routing decisions, applies the capacity
    truncation (first-come, cap = 2560), and builds per-expert gather lists.
  Launch 2 (experts, expert-parallel): core e owns expert e. It gathers its
    tokens' rows from a local replica of x via indirect DMA, transposes them
    on the tensor engine, and runs the SwiGLU expert in bf16
    (y = (silu(x@w1+b1) * (x@w2+b2)) @ w3 + b3) with fp32 PSUM accumulation.
  Host: scatters the per-expert outputs back to token order (rows are
    disjoint for top-1 routing) and assembles the scalar aux loss from the
    device partial sums.
"""

import sys

sys.path.insert(0, "/opt/trn_rl_repo")

import numpy as np

import concourse.bass as bass
import concourse.bacc as bacc
import concourse.bass_isa as bass_isa
import concourse.mybir as mybir
import concourse.tile as tile
from concourse.masks import make_identity
from concourse.bass_utils import run_bass_kernel_spmd

P = 128
B, T, H, F, E = 8, 2048, 1024, 2048, 8
S = B * T
CAP = int(np.ceil(1.25 * S / float(E)))  # 2560
NROWS0 = 2304  # 18 groups of 128; covers the worst expert load with slack
AUX_W, Z_W = 0.01, 0.001

FP32 = mybir.dt.float32
BF16 = mybir.dt.bfloat16
I32 = mybir.dt.int32
U32 = mybir.dt.uint32
AF = mybir.ActivationFunctionType
ALU = mybir.AluOpType

N_CORES = 8
TSHARD = S // N_CORES  # 2048 tokens per core in launch 1
KT_H = H // P  # 8 k-tiles over H
KT_F = F // P  # 16 k-tiles over F

# CoreSim does not implement the Silu activation table; set True to build the
# expert kernel with an equivalent sigmoid(x)*x decomposition (sim testing).
SILU_DECOMPOSE = False


def build_router_nc():
    nc = bacc.Bacc("TRN2", target_bir_lowering=False)
    xs = nc.dram_tensor("xs", [TSHARD, H], FP32, kind="ExternalInput")
    rw = nc.dram_tensor("rw", [P, KT_H * E], FP32, kind="ExternalInput")  # [p, k*8+e]
    rb = nc.dram_tensor("rb", [E], FP32, kind="ExternalInput")
    idx_out = nc.dram_tensor("idx_out", [P, TSHARD // P], U32, kind="ExternalOutput")
    ps_out = nc.dram_tensor("ps_out", [1, E], FP32, kind="ExternalOutput")
    z2_out = nc.dram_tensor("z2_out", [1, 1], FP32, kind="ExternalOutput")

    NT = TSHARD // P  # 16 token tiles

    with tile.TileContext(nc) as tc:
        with (
            tc.tile_pool(name="const", bufs=1) as cpool,
            tc.tile_pool(name="work", bufs=3) as wpool,
            tc.tile_pool(name="acc", bufs=1) as apool,
            tc.tile_pool(name="psum", bufs=2, space="PSUM") as pp,
            tc.tile_pool(name="psum_lg", bufs=2, space="PSUM") as plg,
            tc.tile_pool(name="psum_lg2", bufs=2, space="PSUM") as plg2,
        ):
            ident = cpool.tile([P, P], FP32, tag="ident")
            make_identity(nc, ident[:])
            # first token tile's DMA issued before anything else on the queue
            xs0_sb = wpool.tile([P, H], FP32, tag="xs")
            nc.sync.dma_start(xs0_sb[:], xs[0:P, :])
            rw_sb = cpool.tile([P, KT_H, E], FP32, tag="rw")
            nc.sync.dma_start(rw_sb[:], rw[:].rearrange("p (k e) -> p k e", e=E))
            ones_sb = cpool.tile([1, P], FP32, tag="ones")
            nc.vector.memset(ones_sb[:], 1.0)
            rb_sb = cpool.tile([1, E], FP32, tag="rb")
            nc.sync.dma_start(rb_sb[:], rb[None, :])
            iota8 = cpool.tile([P, E], FP32, tag="iota8")
            iota8i = cpool.tile([P, E], I32, tag="iota8i")
            nc.gpsimd.iota(iota8i[:], [[1, E]], channel_multiplier=0)
            nc.vector.tensor_copy(iota8[:], iota8i[:])

            lgall = apool.tile([P, NT, E], FP32, tag="lgall")

            for t in range(NT):
                if t == 0:
                    xs_sb = xs0_sb
                else:
                    xs_sb = wpool.tile([P, H], FP32, tag="xs")
                    nc.sync.dma_start(xs_sb[:], xs[t * P : (t + 1) * P, :])
                xt_sb = wpool.tile([P, KT_H, P], FP32, tag="xt")
                for k in range(KT_H):
                    tr_ps = pp.tile([P, P], FP32, tag="tr")
                    nc.tensor.transpose(
                        tr_ps[:], xs_sb[:, k * P : (k + 1) * P], ident[:]
                    )
                    nc.vector.tensor_copy(xt_sb[:, k, :], tr_ps[:])
                # logits.T [E, P] with router weights as the stationary operand
                lgT_ps = plg.tile([E, P], FP32, tag="lgT")
                for k in range(KT_H):
                    nc.tensor.matmul(
                        lgT_ps[:],
                        lhsT=rw_sb[:, k, :],
                        rhs=xt_sb[:, k, :],
                        start=(k == 0),
                        stop=False,
                    )
                # += rb.T @ ones : adds router_b[e] to every token's logits
                nc.tensor.matmul(
                    lgT_ps[:],
                    lhsT=rb_sb[0:1, :],
                    rhs=ones_sb[0:1, :],
                    start=False,
                    stop=True,
                )
                lgT_sb = wpool.tile([E, P], FP32, tag="lgT_sb")
                nc.vector.tensor_copy(lgT_sb[:], lgT_ps[:])
                lg_ps2 = plg2.tile([P, E], FP32, tag="lg2")
                nc.tensor.transpose(lg_ps2[:], lgT_sb[:], ident[0:E, 0:E])
                nc.vector.tensor_copy(lgall[:, t, :], lg_ps2[:])

            # batched softmax / logsumexp / argmax stats over all NT tiles
            m16 = apool.tile([P, NT], FP32, tag="m16")
            nc.vector.tensor_reduce(
                m16[:], lgall[:], axis=mybir.AxisListType.X, op=ALU.max
            )
            m16b = m16[:, :, None].to_broadcast([P, NT, E])
            lgc = apool.tile([P, NT, E], FP32, tag="lgc")
            nc.vector.tensor_tensor(lgc[:], lgall[:], m16b, op=ALU.subtract)
            pex = apool.tile([P, NT, E], FP32, tag="pex")
            nc.scalar.activation(
                pex[:].rearrange("p t e -> p (t e)"),
                lgc[:].rearrange("p t e -> p (t e)"),
                AF.Exp,
            )
            se16 = apool.tile([P, NT], FP32, tag="se16")
            nc.vector.tensor_reduce(
                se16[:], pex[:], axis=mybir.AxisListType.X, op=ALU.add
            )
            rse16 = apool.tile([P, NT], FP32, tag="rse16")
            nc.vector.reciprocal(rse16[:], se16[:])
            pn = apool.tile([P, NT, E], FP32, tag="pn")
            nc.vector.tensor_tensor(
                pn[:], pex[:], rse16[:, :, None].to_broadcast([P, NT, E]), op=ALU.mult
            )
            psum8 = apool.tile([P, E], FP32, tag="psum8")
            nc.vector.tensor_reduce(
                psum8[:], pn[:].rearrange("p t e -> p e t"),
                axis=mybir.AxisListType.X, op=ALU.add,
            )
            # argmax per (token): sum(e * (lg == max))
            eq = apool.tile([P, NT, E], FP32, tag="eq")
            nc.vector.tensor_tensor(eq[:], lgall[:], m16b, op=ALU.is_equal)
            nc.vector.tensor_tensor(
                eq[:], eq[:], iota8[:, None, :].to_broadcast([P, NT, E]), op=ALU.mult
            )
            idxf = apool.tile([P, NT], FP32, tag="idxf")
            nc.vector.tensor_reduce(
                idxf[:], eq[:], axis=mybir.AxisListType.X, op=ALU.add
            )
            idxall = apool.tile([P, NT], U32, tag="idxall")
            nc.vector.tensor_copy(idxall[:], idxf[:])

            lnse = apool.tile([P, NT], FP32, tag="lnse")
            nc.scalar.activation(lnse[:], se16[:], AF.Ln)
            zall = apool.tile([P, NT], FP32, tag="zall")
            nc.vector.tensor_tensor(zall[:], lnse[:], m16[:], op=ALU.add)
            z2p = apool.tile([P, 1], FP32, tag="z2p")
            z2sq = apool.tile([P, NT], FP32, tag="z2sq")
            nc.scalar.activation(z2sq[:], zall[:], AF.Square, accum_out=z2p[:])

            z2r = apool.tile([P, 1], FP32, tag="z2r")
            nc.gpsimd.partition_all_reduce(
                z2r[:], z2p[:], channels=P, reduce_op=bass_isa.ReduceOp.add
            )
            pr = apool.tile([P, E], FP32, tag="pr")
            nc.gpsimd.partition_all_reduce(
                pr[:], psum8[:], channels=P, reduce_op=bass_isa.ReduceOp.add
            )
            nc.sync.dma_start(idx_out[:], idxall[:])
            nc.sync.dma_start(ps_out[:], pr[0:1, :])
            nc.sync.dma_start(z2_out[:], z2r[0:1, :])
    nc.finalize()
    return nc


def _chunks(nrows):
    out = []
    pos = 0
    while pos < nrows:
        c = min(1024, nrows - pos)
        out.append((pos, c))
        pos += c
    return out


def build_expert_nc(nrows):
    assert nrows % P == 0
    ng = nrows // P
    nc = bacc.Bacc("TRN2", target_bir_lowering=False)
    xf = nc.dram_tensor("xf", [S, H], FP32, kind="ExternalInput")
    idx = nc.dram_tensor("idx", [P, ng], I32, kind="ExternalInput")
    w1 = nc.dram_tensor("w1", [H, F], FP32, kind="ExternalInput")
    w2 = nc.dram_tensor("w2", [H, F], FP32, kind="ExternalInput")
    w3 = nc.dram_tensor("w3", [F, H], FP32, kind="ExternalInput")
    b1 = nc.dram_tensor("b1", [P, KT_F], FP32, kind="ExternalInput")  # b1[f] at [f%128, f//128]
    b2 = nc.dram_tensor("b2", [P, KT_F], FP32, kind="ExternalInput")
    b3 = nc.dram_tensor("b3", [1, H], FP32, kind="ExternalInput")
    y_out = nc.dram_tensor("y_out", [nrows, H], FP32, kind="ExternalOutput")

    with tile.TileContext(nc) as tc:
        with (
            tc.tile_pool(name="const", bufs=1) as cpool,
            tc.tile_pool(name="wres", bufs=1) as wres,
            tc.tile_pool(name="wstage", bufs=2) as wstage,
            tc.tile_pool(name="xg", bufs=2) as xg,
            tc.tile_pool(name="xgb", bufs=2) as xgb,
            tc.tile_pool(name="xt", bufs=2) as xtp,
            tc.tile_pool(name="gt", bufs=1) as gtp,
            tc.tile_pool(name="ysb", bufs=2) as ysb,
            tc.tile_pool(name="ew", bufs=2) as ew,
            tc.tile_pool(name="ps_tr", bufs=2, space="PSUM") as ps_tr,
            tc.tile_pool(name="ps_h1", bufs=1, space="PSUM") as ps_h1,
            tc.tile_pool(name="ps_h2", bufs=1, space="PSUM") as ps_h2,
            tc.tile_pool(name="ps_y", bufs=2, space="PSUM") as ps_y,
        ):
            ident_bf = cpool.tile([P, P], BF16, tag="identbf")
            make_identity(nc, ident_bf[:])
            idx_sb = cpool.tile([P, ng], I32, tag="idx")
            nc.sync.dma_start(idx_sb[:], idx[:])
            b1_sb = cpool.tile([P, KT_F], FP32, tag="b1")
            nc.sync.dma_start(b1_sb[:], b1[:])
            b2_sb = cpool.tile([P, KT_F], FP32, tag="b2")
            nc.sync.dma_start(b2_sb[:], b2[:])
            b3_row = cpool.tile([1, H], FP32, tag="b3row")
            nc.sync.dma_start(b3_row[:], b3[:])
            b3_bc = cpool.tile([P, H], FP32, tag="b3bc")
            nc.gpsimd.partition_broadcast(b3_bc[:], b3_row[0:1, :])

            def emit_chunk_load(c0, clen):
                ngr = clen // P
                g0 = c0 // P
                xt_sb = xtp.tile([P, KT_H, 1024], BF16, tag="xt")
                for g in range(ngr):
                    xe_sb = xg.tile([P, H], FP32, tag="xe")
                    nc.gpsimd.indirect_dma_start(
                        out=xe_sb[:],
                        out_offset=None,
                        in_=xf[:],
                        in_offset=bass.IndirectOffsetOnAxis(
                            ap=idx_sb[:, g0 + g : g0 + g + 1], axis=0
                        ),
                    )
                    xe_bf = xgb.tile([P, H], BF16, tag="xeb")
                    nc.vector.tensor_copy(xe_bf[:], xe_sb[:])
                    for k in range(KT_H):
                        tr_ps = ps_tr.tile([P, P], BF16, tag="tr")
                        nc.tensor.transpose(
                            tr_ps[:], xe_bf[:, k * P : (k + 1) * P], ident_bf[:]
                        )
                        nc.vector.tensor_copy(
                            xt_sb[:, k, g * P : (g + 1) * P], tr_ps[:]
                        )
                return xt_sb

            chunks = _chunks(nrows)
            # chunk 0 token loads first so the tensor engine warms up early
            xt_first = emit_chunk_load(*chunks[0])

            # resident bf16 weights, staged in consumption order:
            # (w1 c, w2 c) slices for c = 0..3, then w3 (used by the y matmuls)
            w1_bf = wres.tile([P, KT_H, F], BF16, tag="w1")
            w2_bf = wres.tile([P, KT_H, F], BF16, tag="w2")
            w3_bf = wres.tile([P, KT_F, H], BF16, tag="w3")

            def stage(wdram, wbf, k, c):
                st = wstage.tile([P, 512], FP32, tag="wst")
                nc.sync.dma_start(
                    st[:], wdram[k * P : (k + 1) * P, c * 512 : (c + 1) * 512]
                )
                nc.vector.tensor_copy(wbf[:, k, c * 512 : (c + 1) * 512], st[:])

            for c in range(F // 512):
                for k in range(KT_H):
                    stage(w1, w1_bf, k, c)
                for k in range(KT_H):
                    stage(w2, w2_bf, k, c)
            for c in range(H // 512):
                for k in range(KT_F):
                    stage(w3, w3_bf, k, c)

            for ci, (c0, clen) in enumerate(chunks):
                ngr = clen // P
                xt_sb = xt_first if ci == 0 else emit_chunk_load(c0, clen)

                gt_sb = gtp.tile([P, KT_F, 1024], BF16, tag="gt")
                halves = [(h0, min(512, clen - h0)) for h0 in range(0, clen, 512)]
                for ft in range(KT_F):
                    h1_ps = ps_h1.tile([P, 1024], FP32, tag="h1")
                    h2_ps = ps_h2.tile([P, 1024], FP32, tag="h2")
                    for hps, wbf in ((h1_ps, w1_bf), (h2_ps, w2_bf)):
                        for h0, hlen in halves:
                            for k in range(KT_H):
                                nc.tensor.matmul(
                                    hps[:, h0 : h0 + hlen],
                                    lhsT=wbf[:, k, ft * P : (ft + 1) * P],
                                    rhs=xt_sb[:, k, h0 : h0 + hlen],
                                    start=(k == 0),
                                    stop=(k == KT_H - 1),
                                )
                    if SILU_DECOMPOSE:
                        sg = ew.tile([P, 1024], FP32, tag="sg")
                        nc.scalar.activation(
                            sg[:, :clen], h1_ps[:, :clen], AF.Sigmoid,
                            bias=b1_sb[:, ft : ft + 1], scale=1.0,
                        )
                        h1b = ew.tile([P, 1024], FP32, tag="h1b")
                        nc.scalar.activation(
                            h1b[:, :clen], h1_ps[:, :clen], AF.Identity,
                            bias=b1_sb[:, ft : ft + 1], scale=1.0,
                        )
                        s1 = ew.tile([P, 1024], BF16, tag="s1")
                        nc.vector.tensor_tensor(
                            s1[:, :clen], sg[:, :clen], h1b[:, :clen], op=ALU.mult
                        )
                    else:
                        s1 = ew.tile([P, 1024], BF16, tag="s1")
                        nc.scalar.activation(
                            s1[:, :clen], h1_ps[:, :clen], AF.Silu,
                            bias=b1_sb[:, ft : ft + 1], scale=1.0,
                        )
                    h2b = ew.tile([P, 1024], BF16, tag="h2b")
                    nc.scalar.activation(
                        h2b[:, :clen], h2_ps[:, :clen], AF.Identity,
                        bias=b2_sb[:, ft : ft + 1], scale=1.0,
                    )
                    nc.vector.tensor_tensor(
                        gt_sb[:, ft, :clen], s1[:, :clen], h2b[:, :clen], op=ALU.mult
                    )

                for t in range(ngr):
                    y_sb = ysb.tile([P, H], FP32, tag="y")
                    for hc in range(H // 512):
                        y_ps = ps_y.tile([P, 512], FP32, tag="y")
                        for kf in range(KT_F):
                            nc.tensor.matmul(
                                y_ps[:],
                                lhsT=gt_sb[:, kf, t * P : (t + 1) * P],
                                rhs=w3_bf[:, kf, hc * 512 : (hc + 1) * 512],
                                start=(kf == 0),
                                stop=(kf == KT_F - 1),
                            )
                        nc.vector.tensor_tensor(
                            y_sb[:, hc * 512 : (hc + 1) * 512],
                            y_ps[:],
                            b3_bc[:, hc * 512 : (hc + 1) * 512],
                            op=ALU.add,
                        )
                    nc.sync.dma_start(
                        y_out[c0 + t * P : c0 + (t + 1) * P, :], y_sb[:]
                    )
    nc.finalize()
    return nc


_NC_CACHE = {}


def _get_nc(kind, *args):
    key = (kind,) + args
    if key not in _NC_CACHE:
        if kind == "router":
            _NC_CACHE[key] = build_router_nc()
        else:
            _NC_CACHE[key] = build_expert_nc(*args)
    return _NC_CACHE[key]


def _run(nc, in_maps, **kw):
    return run_bass_kernel_spmd(nc, in_maps, core_ids=list(range(N_CORES)), **kw)


def kernel(x, router_w, router_b, w1, b1, w2, b2, w3, b3, _trace=None):
    x = np.ascontiguousarray(np.asarray(x, np.float32))
    router_w = np.ascontiguousarray(np.asarray(router_w, np.float32))
    router_b = np.ascontiguousarray(np.asarray(router_b, np.float32))
    w1 = np.ascontiguousarray(np.asarray(w1, np.float32))
    b1 = np.ascontiguousarray(np.asarray(b1, np.float32))
    w2 = np.ascontiguousarray(np.asarray(w2, np.float32))
    b2 = np.ascontiguousarray(np.asarray(b2, np.float32))
    w3 = np.ascontiguousarray(np.asarray(w3, np.float32))
    b3 = np.ascontiguousarray(np.asarray(b3, np.float32))

    X = x.reshape(S, H)
    # router_w rearranged so SBUF partition p holds rw[k*128+p, e] at [p, k*E+e]
    rw_r = np.ascontiguousarray(
        router_w.reshape(KT_H, P, E).transpose(1, 0, 2).reshape(P, KT_H * E)
    )

    nc1 = _get_nc("router")
    in_maps1 = [
        {
            "xs": X[c * TSHARD : (c + 1) * TSHARD],
            "rw": rw_r,
            "rb": router_b,
        }
        for c in range(N_CORES)
    ]
    res1 = _run(nc1, in_maps1, **({"trace": True, "tmpdir": _trace + "/l1"} if _trace else {}))

    experts = np.empty(S, np.int64)
    ps_total = np.zeros(E, np.float64)
    z2_total = 0.0
    for c in range(N_CORES):
        r = res1.results[c]
        experts[c * TSHARD : (c + 1) * TSHARD] = (
            r["idx_out"].astype(np.int64).T.reshape(TSHARD)
        )
        ps_total += r["ps_out"][0].astype(np.float64)
        z2_total += float(r["z2_out"][0, 0])
    l1_ns = res1.exec_time_ns

    # capacity truncation (drop_policy='first'), gather lists
    kept_lists = []
    load = np.zeros(E, np.float64)
    for e in range(E):
        toks = np.nonzero(experts == e)[0][:CAP]
        kept_lists.append(toks)
        load[e] = len(toks)
    max_kept = max(len(t) for t in kept_lists)
    nrows = NROWS0 if max_kept <= NROWS0 else CAP
    ng = nrows // P

    nc2 = _get_nc("expert", nrows)
    in_maps2 = []
    for e in range(E):
        toks = kept_lists[e]
        padded = np.zeros(nrows, np.int32)
        padded[: len(toks)] = toks
        idx_arr = np.ascontiguousarray(padded.reshape(ng, P).T)
        b1r = np.ascontiguousarray(b1[e].reshape(KT_F, P).T)
        b2r = np.ascontiguousarray(b2[e].reshape(KT_F, P).T)
        in_maps2.append(
            {
                "xf": X,
                "idx": idx_arr,
                "w1": w1[e],
                "w2": w2[e],
                "w3": w3[e],
                "b1": b1r,
                "b2": b2r,
                "b3": b3[e][None, :],
            }
        )
    res2 = _run(nc2, in_maps2, **({"trace": True, "tmpdir": _trace + "/l2"} if _trace else {}))

    out = np.zeros((S, H), np.float32)
    for e in range(E):
        toks = kept_lists[e]
        out[toks] = res2.results[e]["y_out"][: len(toks)]
    l2_ns = res2.exec_time_ns

    importance = ps_total / float(S)
    aux = float((importance * (load / float(S))).sum() * E * AUX_W)
    aux += z2_total / float(S) * Z_W
    aux = np.float32(aux)

    if _trace:
        kernel.last_exec_ns = (l1_ns, l2_ns)
    return out.reshape(B, T, H), aux


# revision 18
# speedup vs baseline: 1.1456x; 1.1130x over previous
"""Trainium2 Bass kernel for a Switch-style top-1 MoE block (BantamMoEBlock).

Strategy (8 NeuronCores, one full TRN2 chip):
  Launch 1 (router, data-parallel): core c takes token shard c (2048 tokens),
    computes logits = x @ router_w + router_b on the tensor engine (fp32),
    softmax stats + top-1 expert id per token, and the partial sums needed
    for the aux loss (sum of probs per expert, sum of logsumexp^2).
  Host: merges the 8 shards' routing decisions, applies the capacity
    truncation (first-come, cap = 2560), and builds per-expert gather lists.
  Launch 2 (experts, expert-parallel): core e owns expert e. It gathers its
    tokens' rows from a local replica of x via indirect DMA, transposes them
    on the tensor engine, and runs the SwiGLU expert in bf16
    (y = (silu(x@w1+b1) * (x@w2+b2)) @ w3 + b3) with fp32 PSUM accumulation.
  Host: scatters the per-expert outputs back to token order (rows are
    disjoint for top-1 routing) and assembles the scalar aux loss from the
    device partial sums.
"""

import sys

sys.path.insert(0, "/opt/trn_rl_repo")

import numpy as np

import concourse.bass as bass
import concourse.bacc as bacc
import concourse.bass_isa as bass_isa
import concourse.mybir as mybir
import concourse.tile as tile
from concourse.masks import make_identity
from concourse.bass_utils import run_bass_kernel_spmd

P = 128
B, T, H, F, E = 8, 2048, 1024, 2048, 8
S = B * T
CAP = int(np.ceil(1.25 * S / float(E)))  # 2560
NROWS0 = 2304  # 18 groups of 128; covers the worst expert load with slack
AUX_W, Z_W = 0.01, 0.001

FP32 = mybir.dt.float32
BF16 = mybir.dt.bfloat16
I32 = mybir.dt.int32
U32 = mybir.dt.uint32
AF = mybir.ActivationFunctionType
ALU = mybir.AluOpType

N_CORES = 8
TSHARD = S // N_CORES  # 2048 tokens per core in launch 1
KT_H = H // P  # 8 k-tiles over H
KT_F = F // P  # 16 k-tiles over F

# CoreSim does not implement the Silu activation table; set True to build the
# expert kernel with an equivalent sigmoid(x)*x decomposition (sim testing).
SILU_DECOMPOSE = False


def build_router_nc():
    nc = bacc.Bacc("TRN2", target_bir_lowering=False)
    xs = nc.dram_tensor("xs", [TSHARD, H], FP32, kind="ExternalInput")
    rw = nc.dram_tensor("rw", [P, KT_H * E], FP32, kind="ExternalInput")  # [p, k*8+e]
    rb = nc.dram_tensor("rb", [E], FP32, kind="ExternalInput")
    idx_out = nc.dram_tensor("idx_out", [P, TSHARD // P], U32, kind="ExternalOutput")
    gap_out = nc.dram_tensor("gap_out", [P, TSHARD // P], FP32, kind="ExternalOutput")
    ps_out = nc.dram_tensor("ps_out", [1, E], FP32, kind="ExternalOutput")
    z2_out = nc.dram_tensor("z2_out", [1, 1], FP32, kind="ExternalOutput")

    NT = TSHARD // P  # 16 token tiles

    with tile.TileContext(nc) as tc:
        with (
            tc.tile_pool(name="const", bufs=1) as cpool,
            tc.tile_pool(name="work", bufs=3) as wpool,
            tc.tile_pool(name="acc", bufs=1) as apool,
            tc.tile_pool(name="psum", bufs=2, space="PSUM") as pp,
            tc.tile_pool(name="psum_lg", bufs=2, space="PSUM") as plg,
            tc.tile_pool(name="psum_lg2", bufs=2, space="PSUM") as plg2,
        ):
            ident = cpool.tile([P, P], FP32, tag="ident")
            make_identity(nc, ident[:])
            ident_bf = cpool.tile([P, P], BF16, tag="identbf")
            make_identity(nc, ident_bf[:])
            # first token tile's DMA issued before anything else on the queue
            xs0_sb = wpool.tile([P, H], FP32, tag="xs")
            nc.sync.dma_start(xs0_sb[:], xs[0:P, :])
            rw_sb = cpool.tile([P, KT_H, E], FP32, tag="rw")
            nc.sync.dma_start(rw_sb[:], rw[:].rearrange("p (k e) -> p k e", e=E))
            rw_bf = cpool.tile([P, KT_H, E], BF16, tag="rwbf")
            nc.vector.tensor_copy(rw_bf[:], rw_sb[:])
            ones_bf = cpool.tile([1, P], BF16, tag="ones")
            nc.vector.memset(ones_bf[:], 1.0)
            rb_sb = cpool.tile([1, E], FP32, tag="rb")
            nc.sync.dma_start(rb_sb[:], rb[None, :])
            rb_bf = cpool.tile([1, E], BF16, tag="rbbf")
            nc.vector.tensor_copy(rb_bf[:], rb_sb[:])
            iota8 = cpool.tile([P, E], FP32, tag="iota8")
            iota8i = cpool.tile([P, E], I32, tag="iota8i")
            nc.gpsimd.iota(iota8i[:], [[1, E]], channel_multiplier=0)
            nc.vector.tensor_copy(iota8[:], iota8i[:])

            lgall = apool.tile([P, NT, E], FP32, tag="lgall")

            for t in range(NT):
                if t == 0:
                    xs_sb = xs0_sb
                else:
                    xs_sb = wpool.tile([P, H], FP32, tag="xs")
                    nc.sync.dma_start(xs_sb[:], xs[t * P : (t + 1) * P, :])
                xs_bf = wpool.tile([P, H], BF16, tag="xsbf")
                nc.vector.tensor_copy(xs_bf[:], xs_sb[:])
                xt_bf = wpool.tile([P, KT_H, P], BF16, tag="xt")
                for k in range(KT_H):
                    tr_ps = pp.tile([P, P], BF16, tag="tr")
                    nc.tensor.transpose(
                        tr_ps[:], xs_bf[:, k * P : (k + 1) * P], ident_bf[:]
                    )
                    nc.vector.tensor_copy(xt_bf[:, k, :], tr_ps[:])
                # logits.T [E, P] with router weights as the stationary operand
                lgT_ps = plg.tile([E, P], FP32, tag="lgT")
                for k in range(KT_H):
                    nc.tensor.matmul(
                        lgT_ps[:],
                        lhsT=rw_bf[:, k, :],
                        rhs=xt_bf[:, k, :],
                        start=(k == 0),
                        stop=False,
                    )
                # += rb.T @ ones : adds router_b[e] to every token's logits
                nc.tensor.matmul(
                    lgT_ps[:],
                    lhsT=rb_bf[0:1, :],
                    rhs=ones_bf[0:1, :],
                    start=False,
                    stop=True,
                )
                lgT_sb = wpool.tile([E, P], FP32, tag="lgT_sb")
                nc.vector.tensor_copy(lgT_sb[:], lgT_ps[:])
                lg_ps2 = plg2.tile([P, E], FP32, tag="lg2")
                nc.tensor.transpose(lg_ps2[:], lgT_sb[:], ident[0:E, 0:E])
                nc.vector.tensor_copy(lgall[:, t, :], lg_ps2[:])

            # batched softmax / logsumexp / argmax stats over all NT tiles
            m16 = apool.tile([P, NT], FP32, tag="m16")
            nc.vector.tensor_reduce(
                m16[:], lgall[:], axis=mybir.AxisListType.X, op=ALU.max
            )
            m16b = m16[:, :, None].to_broadcast([P, NT, E])
            lgc = apool.tile([P, NT, E], FP32, tag="lgc")
            nc.vector.tensor_tensor(lgc[:], lgall[:], m16b, op=ALU.subtract)
            pex = apool.tile([P, NT, E], FP32, tag="pex")
            nc.scalar.activation(
                pex[:].rearrange("p t e -> p (t e)"),
                lgc[:].rearrange("p t e -> p (t e)"),
                AF.Exp,
            )
            se16 = apool.tile([P, NT], FP32, tag="se16")
            nc.vector.tensor_reduce(
                se16[:], pex[:], axis=mybir.AxisListType.X, op=ALU.add
            )
            rse16 = apool.tile([P, NT], FP32, tag="rse16")
            nc.vector.reciprocal(rse16[:], se16[:])
            pn = apool.tile([P, NT, E], FP32, tag="pn")
            nc.vector.tensor_tensor(
                pn[:], pex[:], rse16[:, :, None].to_broadcast([P, NT, E]), op=ALU.mult
            )
            psum8 = apool.tile([P, E], FP32, tag="psum8")
            nc.vector.tensor_reduce(
                psum8[:], pn[:].rearrange("p t e -> p e t"),
                axis=mybir.AxisListType.X, op=ALU.add,
            )
            # argmax per (token): sum(e * (lg == max)); top-2 gap for the host
            eq = apool.tile([P, NT, E], FP32, tag="eq")
            nc.vector.tensor_tensor(eq[:], lgall[:], m16b, op=ALU.is_equal)
            lg2nd = apool.tile([P, NT, E], FP32, tag="lg2nd")
            nc.vector.tensor_scalar_mul(lg2nd[:], eq[:], -1e30)
            nc.vector.tensor_tensor(lg2nd[:], lg2nd[:], lgall[:], op=ALU.add)
            m2 = apool.tile([P, NT], FP32, tag="m2")
            nc.vector.tensor_reduce(
                m2[:], lg2nd[:], axis=mybir.AxisListType.X, op=ALU.max
            )
            gap16 = apool.tile([P, NT], FP32, tag="gap16")
            nc.vector.tensor_tensor(gap16[:], m16[:], m2[:], op=ALU.subtract)
            nc.vector.tensor_tensor(
                eq[:], eq[:], iota8[:, None, :].to_broadcast([P, NT, E]), op=ALU.mult
            )
            idxf = apool.tile([P, NT], FP32, tag="idxf")
            nc.vector.tensor_reduce(
                idxf[:], eq[:], axis=mybir.AxisListType.X, op=ALU.add
            )
            idxall = apool.tile([P, NT], U32, tag="idxall")
            nc.vector.tensor_copy(idxall[:], idxf[:])

            lnse = apool.tile([P, NT], FP32, tag="lnse")
            nc.scalar.activation(lnse[:], se16[:], AF.Ln)
            zall = apool.tile([P, NT], FP32, tag="zall")
            nc.vector.tensor_tensor(zall[:], lnse[:], m16[:], op=ALU.add)
            z2p = apool.tile([P, 1], FP32, tag="z2p")
            z2sq = apool.tile([P, NT], FP32, tag="z2sq")
            nc.scalar.activation(z2sq[:], zall[:], AF.Square, accum_out=z2p[:])

            z2r = apool.tile([P, 1], FP32, tag="z2r")
            nc.gpsimd.partition_all_reduce(
                z2r[:], z2p[:], channels=P, reduce_op=bass_isa.ReduceOp.add
            )
            pr = apool.tile([P, E], FP32, tag="pr")
            nc.gpsimd.partition_all_reduce(
                pr[:], psum8[:], channels=P, reduce_op=bass_isa.ReduceOp.add
            )
            nc.sync.dma_start(idx_out[:], idxall[:])
            nc.sync.dma_start(gap_out[:], gap16[:])
            nc.sync.dma_start(ps_out[:], pr[0:1, :])
            nc.sync.dma_start(z2_out[:], z2r[0:1, :])
    nc.finalize()
    return nc


def _chunks(nrows):
    out = []
    pos = 0
    while pos < nrows:
        c = min(1024, nrows - pos)
        out.append((pos, c))
        pos += c
    return out


def build_expert_nc(nrows):
    assert nrows % P == 0
    ng = nrows // P
    nc = bacc.Bacc("TRN2", target_bir_lowering=False)
    xf = nc.dram_tensor("xf", [S, H], FP32, kind="ExternalInput")
    idx = nc.dram_tensor("idx", [P, ng], I32, kind="ExternalInput")
    w1 = nc.dram_tensor("w1", [H, F], FP32, kind="ExternalInput")
    w2 = nc.dram_tensor("w2", [H, F], FP32, kind="ExternalInput")
    w3 = nc.dram_tensor("w3", [F, H], FP32, kind="ExternalInput")
    b1 = nc.dram_tensor("b1", [P, KT_F], FP32, kind="ExternalInput")  # b1[f] at [f%128, f//128]
    b2 = nc.dram_tensor("b2", [P, KT_F], FP32, kind="ExternalInput")
    b3 = nc.dram_tensor("b3", [1, H], FP32, kind="ExternalInput")
    y_out = nc.dram_tensor("y_out", [nrows, H], FP32, kind="ExternalOutput")

    with tile.TileContext(nc) as tc:
        with (
            tc.tile_pool(name="const", bufs=1) as cpool,
            tc.tile_pool(name="wres", bufs=1) as wres,
            tc.tile_pool(name="wstage", bufs=2) as wstage,
            tc.tile_pool(name="xg", bufs=2) as xg,
            tc.tile_pool(name="xgb", bufs=2) as xgb,
            tc.tile_pool(name="xt", bufs=2) as xtp,
            tc.tile_pool(name="gt", bufs=1) as gtp,
            tc.tile_pool(name="ysb", bufs=2) as ysb,
            tc.tile_pool(name="ew", bufs=2) as ew,
            tc.tile_pool(name="ps_tr", bufs=2, space="PSUM") as ps_tr,
            tc.tile_pool(name="ps_h1", bufs=1, space="PSUM") as ps_h1,
            tc.tile_pool(name="ps_h2", bufs=1, space="PSUM") as ps_h2,
            tc.tile_pool(name="ps_y", bufs=2, space="PSUM") as ps_y,
        ):
            ident_bf = cpool.tile([P, P], BF16, tag="identbf")
            make_identity(nc, ident_bf[:])
            idx_sb = cpool.tile([P, ng], I32, tag="idx")
            nc.sync.dma_start(idx_sb[:], idx[:])
            b1_sb = cpool.tile([P, KT_F], FP32, tag="b1")
            nc.sync.dma_start(b1_sb[:], b1[:])
            b2_sb = cpool.tile([P, KT_F], FP32, tag="b2")
            nc.sync.dma_start(b2_sb[:], b2[:])
            b3_row = cpool.tile([1, H], FP32, tag="b3row")
            nc.sync.dma_start(b3_row[:], b3[:])
            b3_bc = cpool.tile([P, H], FP32, tag="b3bc")
            nc.gpsimd.partition_broadcast(b3_bc[:], b3_row[0:1, :])

            def emit_chunk_load(c0, clen):
                ngr = clen // P
                g0 = c0 // P
                xt_sb = xtp.tile([P, KT_H, 1024], BF16, tag="xt")
                for g in range(ngr):
                    xe_sb = xg.tile([P, H], FP32, tag="xe")
                    nc.gpsimd.indirect_dma_start(
                        out=xe_sb[:],
                        out_offset=None,
                        in_=xf[:],
                        in_offset=bass.IndirectOffsetOnAxis(
                            ap=idx_sb[:, g0 + g : g0 + g + 1], axis=0
                        ),
                    )
                    xe_bf = xgb.tile([P, H], BF16, tag="xeb")
                    nc.vector.tensor_copy(xe_bf[:], xe_sb[:])
                    for k in range(KT_H):
                        tr_ps = ps_tr.tile([P, P], BF16, tag="tr")
                        nc.tensor.transpose(
                            tr_ps[:], xe_bf[:, k * P : (k + 1) * P], ident_bf[:]
                        )
                        nc.vector.tensor_copy(
                            xt_sb[:, k, g * P : (g + 1) * P], tr_ps[:]
                        )
                return xt_sb

            chunks = _chunks(nrows)
            # chunk 0 token loads first so the tensor engine warms up early
            xt_first = emit_chunk_load(*chunks[0])

            # resident bf16 weights, staged in consumption order:
            # (w1 c, w2 c) slices for c = 0..3, then w3 (used by the y matmuls)
            w1_bf = wres.tile([P, KT_H, F], BF16, tag="w1")
            w2_bf = wres.tile([P, KT_H, F], BF16, tag="w2")
            w3_bf = wres.tile([P, KT_F, H], BF16, tag="w3")

            def stage(wdram, wbf, k, c):
                st = wstage.tile([P, 512], FP32, tag="wst")
                nc.sync.dma_start(
                    st[:], wdram[k * P : (k + 1) * P, c * 512 : (c + 1) * 512]
                )
                nc.vector.tensor_copy(wbf[:, k, c * 512 : (c + 1) * 512], st[:])

            for c in range(F // 512):
                for k in range(KT_H):
                    stage(w1, w1_bf, k, c)
                for k in range(KT_H):
                    stage(w2, w2_bf, k, c)
            for c in range(H // 512):
                for k in range(KT_F):
                    stage(w3, w3_bf, k, c)

            for ci, (c0, clen) in enumerate(chunks):
                ngr = clen // P
                xt_sb = xt_first if ci == 0 else emit_chunk_load(c0, clen)

                gt_sb = gtp.tile([P, KT_F, 1024], BF16, tag="gt")
                halves = [(h0, min(512, clen - h0)) for h0 in range(0, clen, 512)]
                for ft in range(KT_F):
                    h1_ps = ps_h1.tile([P, 1024], FP32, tag="h1")
                    h2_ps = ps_h2.tile([P, 1024], FP32, tag="h2")
                    for hps, wbf in ((h1_ps, w1_bf), (h2_ps, w2_bf)):
                        for h0, hlen in halves:
                            for k in range(KT_H):
                                nc.tensor.matmul(
                                    hps[:, h0 : h0 + hlen],
                                    lhsT=wbf[:, k, ft * P : (ft + 1) * P],
                                    rhs=xt_sb[:, k, h0 : h0 + hlen],
                                    start=(k == 0),
                                    stop=(k == KT_H - 1),
                                )
                    if SILU_DECOMPOSE:
                        sg = ew.tile([P, 1024], FP32, tag="sg")
                        nc.scalar.activation(
                            sg[:, :clen], h1_ps[:, :clen], AF.Sigmoid,
                            bias=b1_sb[:, ft : ft + 1], scale=1.0,
                        )
                        h1b = ew.tile([P, 1024], FP32, tag="h1b")
                        nc.scalar.activation(
                            h1b[:, :clen], h1_ps[:, :clen], AF.Identity,
                            bias=b1_sb[:, ft : ft + 1], scale=1.0,
                        )
                        s1 = ew.tile([P, 1024], BF16, tag="s1")
                        nc.vector.tensor_tensor(
                            s1[:, :clen], sg[:, :clen], h1b[:, :clen], op=ALU.mult
                        )
                    else:
                        s1 = ew.tile([P, 1024], BF16, tag="s1")
                        nc.scalar.activation(
                            s1[:, :clen], h1_ps[:, :clen], AF.Silu,
                            bias=b1_sb[:, ft : ft + 1], scale=1.0,
                        )
                    h2b = ew.tile([P, 1024], BF16, tag="h2b")
                    nc.scalar.activation(
                        h2b[:, :clen], h2_ps[:, :clen], AF.Identity,
                        bias=b2_sb[:, ft : ft + 1], scale=1.0,
                    )
                    nc.vector.tensor_tensor(
                        gt_sb[:, ft, :clen], s1[:, :clen], h2b[:, :clen], op=ALU.mult
                    )

                for t in range(ngr):
                    y_sb = ysb.tile([P, H], FP32, tag="y")
                    for hc in range(H // 512):
                        y_ps = ps_y.tile([P, 512], FP32, tag="y")
                        for kf in range(KT_F):
                            nc.tensor.matmul(
                                y_ps[:],
                                lhsT=gt_sb[:, kf, t * P : (t + 1) * P],
                                rhs=w3_bf[:, kf, hc * 512 : (hc + 1) * 512],
                                start=(kf == 0),
                                stop=(kf == KT_F - 1),
                            )
                        nc.vector.tensor_tensor(
                            y_sb[:, hc * 512 : (hc + 1) * 512],
                            y_ps[:],
                            b3_bc[:, hc * 512 : (hc + 1) * 512],
                            op=ALU.add,
                        )
                    nc.sync.dma_start(
                        y_out[c0 + t * P : c0 + (t + 1) * P, :], y_sb[:]
                    )
    nc.finalize()
    return nc


_NC_CACHE = {}


def _get_nc(kind, *args):
    key = (kind,) + args
    if key not in _NC_CACHE:
        if kind == "router":
            _NC_CACHE[key] = build_router_nc()
        else:
            _NC_CACHE[key] = build_expert_nc(*args)
    return _NC_CACHE[key]


def _run(nc, in_maps, **kw):
    return run_bass_kernel_spmd(nc, in_maps, core_ids=list(range(N_CORES)), **kw)


def kernel(x, router_w, router_b, w1, b1, w2, b2, w3, b3, _trace=None):
    x = np.ascontiguousarray(np.asarray(x, np.float32))
    router_w = np.ascontiguousarray(np.asarray(router_w, np.float32))
    router_b = np.ascontiguousarray(np.asarray(router_b, np.float32))
    w1 = np.ascontiguousarray(np.asarray(w1, np.float32))
    b1 = np.ascontiguousarray(np.asarray(b1, np.float32))
    w2 = np.ascontiguousarray(np.asarray(w2, np.float32))
    b2 = np.ascontiguousarray(np.asarray(b2, np.float32))
    w3 = np.ascontiguousarray(np.asarray(w3, np.float32))
    b3 = np.ascontiguousarray(np.asarray(b3, np.float32))

    X = x.reshape(S, H)
    # router_w rearranged so SBUF partition p holds rw[k*128+p, e] at [p, k*E+e]
    rw_r = np.ascontiguousarray(
        router_w.reshape(KT_H, P, E).transpose(1, 0, 2).reshape(P, KT_H * E)
    )

    nc1 = _get_nc("router")
    in_maps1 = [
        {
            "xs": X[c * TSHARD : (c + 1) * TSHARD],
            "rw": rw_r,
            "rb": router_b,
        }
        for c in range(N_CORES)
    ]
    res1 = _run(nc1, in_maps1, **({"trace": True, "tmpdir": _trace + "/l1"} if _trace else {}))

    experts = np.empty(S, np.int64)
    gaps = np.empty(S, np.float64)
    ps_total = np.zeros(E, np.float64)
    z2_total = 0.0
    for c in range(N_CORES):
        r = res1.results[c]
        experts[c * TSHARD : (c + 1) * TSHARD] = (
            r["idx_out"].astype(np.int64).T.reshape(TSHARD)
        )
        gaps[c * TSHARD : (c + 1) * TSHARD] = (
            r["gap_out"].astype(np.float64).T.reshape(TSHARD)
        )
        ps_total += r["ps_out"][0].astype(np.float64)
        z2_total += float(r["z2_out"][0, 0])
    l1_ns = res1.exec_time_ns

    # The device router runs in bf16; its top-1 choice is trusted only when the
    # top-2 gap clears a margin that bounds the worst-case bf16 logit error.
    # Near-ties are re-decided from the exact inputs.
    amb = np.nonzero(gaps < 0.06)[0]
    if amb.size:
        lg_fix = X[amb].astype(np.float64) @ router_w.astype(np.float64) + router_b
        experts[amb] = np.argmax(lg_fix, axis=1)

    # capacity truncation (drop_policy='first'), gather lists
    kept_lists = []
    load = np.zeros(E, np.float64)
    for e in range(E):
        toks = np.nonzero(experts == e)[0][:CAP]
        kept_lists.append(toks)
        load[e] = len(toks)
    max_kept = max(len(t) for t in kept_lists)
    nrows = NROWS0 if max_kept <= NROWS0 else CAP
    ng = nrows // P

    nc2 = _get_nc("expert", nrows)
    in_maps2 = []
    for e in range(E):
        toks = kept_lists[e]
        padded = np.zeros(nrows, np.int32)
        padded[: len(toks)] = toks
        idx_arr = np.ascontiguousarray(padded.reshape(ng, P).T)
        b1r = np.ascontiguousarray(b1[e].reshape(KT_F, P).T)
        b2r = np.ascontiguousarray(b2[e].reshape(KT_F, P).T)
        in_maps2.append(
            {
                "xf": X,
                "idx": idx_arr,
                "w1": w1[e],
                "w2": w2[e],
                "w3": w3[e],
                "b1": b1r,
                "b2": b2r,
                "b3": b3[e][None, :],
            }
        )
    res2 = _run(nc2, in_maps2, **({"trace": True, "tmpdir": _trace + "/l2"} if _trace else {}))

    out = np.zeros((S, H), np.float32)
    for e in range(E):
        toks = kept_lists[e]
        out[toks] = res2.results[e]["y_out"][: len(toks)]
    l2_ns = res2.exec_time_ns

    importance = ps_total / float(S)
    aux = float((importance * (load / float(S))).sum() * E * AUX_W)
    aux += z2_total / float(S) * Z_W
    aux = np.float32(aux)

    if _trace:
        kernel.last_exec_ns = (l1_ns, l2_ns)
    return out.reshape(B, T, H), aux


# revision 22
# speedup vs baseline: 1.1868x; 1.0360x over previous
"""Trainium2 Bass kernel for a Switch-style top-1 MoE block (BantamMoEBlock).

Strategy (8 NeuronCores, one full TRN2 chip):
  Launch 1 (router, data-parallel): core c takes token shard c (2048 tokens),
    computes logits = x @ router_w + router_b on the tensor engine (fp32),
    softmax stats + top-1 expert id per token, and the partial sums needed
    for the aux loss (sum of probs per expert, sum of logsumexp^2).
  Host: merges the 8 shards' routing decisions, applies the capacity
    truncation (first-come, cap = 2560), and builds per-expert gather lists.
  Launch 2 (experts, expert-parallel): core e owns expert e. It gathers its
    tokens' rows from a local replica of x via indirect DMA, transposes them
    on the tensor engine, and runs the SwiGLU expert in bf16
    (y = (silu(x@w1+b1) * (x@w2+b2)) @ w3 + b3) with fp32 PSUM accumulation.
  Host: scatters the per-expert outputs back to token order (rows are
    disjoint for top-1 routing) and assembles the scalar aux loss from the
    device partial sums.
"""

import sys

sys.path.insert(0, "/opt/trn_rl_repo")

import numpy as np

import concourse.bass as bass
import concourse.bacc as bacc
import concourse.bass_isa as bass_isa
import concourse.mybir as mybir
import concourse.tile as tile
from concourse.masks import make_identity
from concourse.bass_utils import run_bass_kernel_spmd

P = 128
B, T, H, F, E = 8, 2048, 1024, 2048, 8
S = B * T
CAP = int(np.ceil(1.25 * S / float(E)))  # 2560
NROWS0 = 2304  # 18 groups of 128; covers the worst expert load with slack
AUX_W, Z_W = 0.01, 0.001

FP32 = mybir.dt.float32
BF16 = mybir.dt.bfloat16
I32 = mybir.dt.int32
U32 = mybir.dt.uint32
AF = mybir.ActivationFunctionType
ALU = mybir.AluOpType

N_CORES = 8
TSHARD = S // N_CORES  # 2048 tokens per core in launch 1
KT_H = H // P  # 8 k-tiles over H
KT_F = F // P  # 16 k-tiles over F

# CoreSim does not implement the Silu activation table; set True to build the
# expert kernel with an equivalent sigmoid(x)*x decomposition (sim testing).
SILU_DECOMPOSE = False


def build_router_nc():
    nc = bacc.Bacc("TRN2", target_bir_lowering=False)
    xs = nc.dram_tensor("xs", [TSHARD, H], FP32, kind="ExternalInput")
    rw = nc.dram_tensor("rw", [P, KT_H * E], FP32, kind="ExternalInput")  # [p, k*8+e]
    rb = nc.dram_tensor("rb", [E], FP32, kind="ExternalInput")
    idx_out = nc.dram_tensor("idx_out", [P, TSHARD // P], U32, kind="ExternalOutput")
    gap_out = nc.dram_tensor("gap_out", [P, TSHARD // P], FP32, kind="ExternalOutput")
    ps_out = nc.dram_tensor("ps_out", [1, E], FP32, kind="ExternalOutput")
    z2_out = nc.dram_tensor("z2_out", [1, 1], FP32, kind="ExternalOutput")

    NT = TSHARD // P  # 16 token tiles

    with tile.TileContext(nc) as tc:
        with (
            tc.tile_pool(name="const", bufs=1) as cpool,
            tc.tile_pool(name="work", bufs=3) as wpool,
            tc.tile_pool(name="acc", bufs=1) as apool,
            tc.tile_pool(name="psum", bufs=2, space="PSUM") as pp,
            tc.tile_pool(name="psum_lg", bufs=2, space="PSUM") as plg,
            tc.tile_pool(name="psum_lg2", bufs=2, space="PSUM") as plg2,
        ):
            ident = cpool.tile([P, P], FP32, tag="ident")
            make_identity(nc, ident[:])
            ident_bf = cpool.tile([P, P], BF16, tag="identbf")
            make_identity(nc, ident_bf[:])
            # first token tile's DMA issued before anything else on the queue
            xs0_sb = wpool.tile([P, H], FP32, tag="xs")
            nc.sync.dma_start(xs0_sb[:], xs[0:P, :])
            rw_sb = cpool.tile([P, KT_H, E], FP32, tag="rw")
            nc.sync.dma_start(rw_sb[:], rw[:].rearrange("p (k e) -> p k e", e=E))
            rw_bf = cpool.tile([P, KT_H, E], BF16, tag="rwbf")
            nc.vector.tensor_copy(rw_bf[:], rw_sb[:])
            ones_bf = cpool.tile([1, P], BF16, tag="ones")
            nc.vector.memset(ones_bf[:], 1.0)
            rb_sb = cpool.tile([1, E], FP32, tag="rb")
            nc.sync.dma_start(rb_sb[:], rb[None, :])
            rb_bf = cpool.tile([1, E], BF16, tag="rbbf")
            nc.vector.tensor_copy(rb_bf[:], rb_sb[:])
            iota8 = cpool.tile([P, E], FP32, tag="iota8")
            iota8i = cpool.tile([P, E], I32, tag="iota8i")
            nc.gpsimd.iota(iota8i[:], [[1, E]], channel_multiplier=0)
            nc.vector.tensor_copy(iota8[:], iota8i[:])

            lgall = apool.tile([P, NT, E], FP32, tag="lgall")

            for t in range(NT):
                if t == 0:
                    xs_sb = xs0_sb
                else:
                    xs_sb = wpool.tile([P, H], FP32, tag="xs")
                    nc.sync.dma_start(xs_sb[:], xs[t * P : (t + 1) * P, :])
                xs_bf = wpool.tile([P, H], BF16, tag="xsbf")
                nc.vector.tensor_copy(xs_bf[:], xs_sb[:])
                xt_bf = wpool.tile([P, KT_H, P], BF16, tag="xt")
                for k in range(KT_H):
                    tr_ps = pp.tile([P, P], BF16, tag="tr")
                    nc.tensor.transpose(
                        tr_ps[:], xs_bf[:, k * P : (k + 1) * P], ident_bf[:]
                    )
                    nc.vector.tensor_copy(xt_bf[:, k, :], tr_ps[:])
                # logits.T [E, P] with router weights as the stationary operand
                lgT_ps = plg.tile([E, P], FP32, tag="lgT")
                for k in range(KT_H):
                    nc.tensor.matmul(
                        lgT_ps[:],
                        lhsT=rw_bf[:, k, :],
                        rhs=xt_bf[:, k, :],
                        start=(k == 0),
                        stop=False,
                    )
                # += rb.T @ ones : adds router_b[e] to every token's logits
                nc.tensor.matmul(
                    lgT_ps[:],
                    lhsT=rb_bf[0:1, :],
                    rhs=ones_bf[0:1, :],
                    start=False,
                    stop=True,
                )
                lgT_sb = wpool.tile([E, P], FP32, tag="lgT_sb")
                nc.vector.tensor_copy(lgT_sb[:], lgT_ps[:])
                lg_ps2 = plg2.tile([P, E], FP32, tag="lg2")
                nc.tensor.transpose(lg_ps2[:], lgT_sb[:], ident[0:E, 0:E])
                nc.vector.tensor_copy(lgall[:, t, :], lg_ps2[:])

            # batched softmax / logsumexp / argmax stats over all NT tiles
            m16 = apool.tile([P, NT], FP32, tag="m16")
            nc.vector.tensor_reduce(
                m16[:], lgall[:], axis=mybir.AxisListType.X, op=ALU.max
            )
            m16b = m16[:, :, None].to_broadcast([P, NT, E])
            lgc = apool.tile([P, NT, E], FP32, tag="lgc")
            nc.vector.tensor_tensor(lgc[:], lgall[:], m16b, op=ALU.subtract)
            pex = apool.tile([P, NT, E], FP32, tag="pex")
            nc.scalar.activation(
                pex[:].rearrange("p t e -> p (t e)"),
                lgc[:].rearrange("p t e -> p (t e)"),
                AF.Exp,
            )
            se16 = apool.tile([P, NT], FP32, tag="se16")
            nc.vector.tensor_reduce(
                se16[:], pex[:], axis=mybir.AxisListType.X, op=ALU.add
            )
            rse16 = apool.tile([P, NT], FP32, tag="rse16")
            nc.vector.reciprocal(rse16[:], se16[:])
            pn = apool.tile([P, NT, E], FP32, tag="pn")
            nc.vector.tensor_tensor(
                pn[:], pex[:], rse16[:, :, None].to_broadcast([P, NT, E]), op=ALU.mult
            )
            psum8 = apool.tile([P, E], FP32, tag="psum8")
            nc.vector.tensor_reduce(
                psum8[:], pn[:].rearrange("p t e -> p e t"),
                axis=mybir.AxisListType.X, op=ALU.add,
            )
            # argmax per (token): sum(e * (lg == max)); top-2 gap for the host
            eq = apool.tile([P, NT, E], FP32, tag="eq")
            nc.vector.tensor_tensor(eq[:], lgall[:], m16b, op=ALU.is_equal)
            lg2nd = apool.tile([P, NT, E], FP32, tag="lg2nd")
            nc.vector.tensor_scalar_mul(lg2nd[:], eq[:], -1e30)
            nc.vector.tensor_tensor(lg2nd[:], lg2nd[:], lgall[:], op=ALU.add)
            m2 = apool.tile([P, NT], FP32, tag="m2")
            nc.vector.tensor_reduce(
                m2[:], lg2nd[:], axis=mybir.AxisListType.X, op=ALU.max
            )
            gap16 = apool.tile([P, NT], FP32, tag="gap16")
            nc.vector.tensor_tensor(gap16[:], m16[:], m2[:], op=ALU.subtract)
            nc.vector.tensor_tensor(
                eq[:], eq[:], iota8[:, None, :].to_broadcast([P, NT, E]), op=ALU.mult
            )
            idxf = apool.tile([P, NT], FP32, tag="idxf")
            nc.vector.tensor_reduce(
                idxf[:], eq[:], axis=mybir.AxisListType.X, op=ALU.add
            )
            idxall = apool.tile([P, NT], U32, tag="idxall")
            nc.vector.tensor_copy(idxall[:], idxf[:])

            lnse = apool.tile([P, NT], FP32, tag="lnse")
            nc.scalar.activation(lnse[:], se16[:], AF.Ln)
            zall = apool.tile([P, NT], FP32, tag="zall")
            nc.vector.tensor_tensor(zall[:], lnse[:], m16[:], op=ALU.add)
            z2p = apool.tile([P, 1], FP32, tag="z2p")
            z2sq = apool.tile([P, NT], FP32, tag="z2sq")
            nc.scalar.activation(z2sq[:], zall[:], AF.Square, accum_out=z2p[:])

            z2r = apool.tile([P, 1], FP32, tag="z2r")
            nc.gpsimd.partition_all_reduce(
                z2r[:], z2p[:], channels=P, reduce_op=bass_isa.ReduceOp.add
            )
            pr = apool.tile([P, E], FP32, tag="pr")
            nc.gpsimd.partition_all_reduce(
                pr[:], psum8[:], channels=P, reduce_op=bass_isa.ReduceOp.add
            )
            nc.sync.dma_start(idx_out[:], idxall[:])
            nc.sync.dma_start(gap_out[:], gap16[:])
            nc.sync.dma_start(ps_out[:], pr[0:1, :])
            nc.sync.dma_start(z2_out[:], z2r[0:1, :])
    nc.finalize()
    return nc


def _chunks(nrows):
    out = []
    pos = 0
    while pos < nrows:
        c = min(1024, nrows - pos)
        out.append((pos, c))
        pos += c
    return out


def build_expert_nc(nrows):
    assert nrows % P == 0
    ng = nrows // P
    nc = bacc.Bacc("TRN2", target_bir_lowering=False)
    xf = nc.dram_tensor("xf", [S, H], FP32, kind="ExternalInput")
    idx = nc.dram_tensor("idx", [P, ng], I32, kind="ExternalInput")
    w1 = nc.dram_tensor("w1", [H, F], FP32, kind="ExternalInput")
    w2 = nc.dram_tensor("w2", [H, F], FP32, kind="ExternalInput")
    w3 = nc.dram_tensor("w3", [F, H], FP32, kind="ExternalInput")
    b1 = nc.dram_tensor("b1", [P, KT_F], FP32, kind="ExternalInput")  # b1[f] at [f%128, f//128]
    b2 = nc.dram_tensor("b2", [P, KT_F], FP32, kind="ExternalInput")
    b3 = nc.dram_tensor("b3", [1, H], FP32, kind="ExternalInput")
    y_out = nc.dram_tensor("y_out", [nrows, H], FP32, kind="ExternalOutput")

    with tile.TileContext(nc) as tc:
        with (
            tc.tile_pool(name="const", bufs=1) as cpool,
            tc.tile_pool(name="wres", bufs=1) as wres,
            tc.tile_pool(name="wstage", bufs=2) as wstage,
            tc.tile_pool(name="xg", bufs=2) as xg,
            tc.tile_pool(name="xgb", bufs=2) as xgb,
            tc.tile_pool(name="xt", bufs=2) as xtp,
            tc.tile_pool(name="gt", bufs=1) as gtp,
            tc.tile_pool(name="ysb", bufs=2) as ysb,
            tc.tile_pool(name="ew", bufs=2) as ew,
            tc.tile_pool(name="ps_tr", bufs=2, space="PSUM") as ps_tr,
            tc.tile_pool(name="ps_h1", bufs=1, space="PSUM") as ps_h1,
            tc.tile_pool(name="ps_h2", bufs=1, space="PSUM") as ps_h2,
            tc.tile_pool(name="ps_y", bufs=2, space="PSUM") as ps_y,
        ):
            ident_bf = cpool.tile([P, P], BF16, tag="identbf")
            make_identity(nc, ident_bf[:])
            idx_sb = cpool.tile([P, ng], I32, tag="idx")
            nc.sync.dma_start(idx_sb[:], idx[:])

            def emit_chunk_load(c0, clen):
                ngr = clen // P
                g0 = c0 // P
                xt_sb = xtp.tile([P, KT_H, 1024], BF16, tag="xt")
                for g in range(ngr):
                    xe_sb = xg.tile([P, H], FP32, tag="xe")
                    nc.gpsimd.indirect_dma_start(
                        out=xe_sb[:],
                        out_offset=None,
                        in_=xf[:],
                        in_offset=bass.IndirectOffsetOnAxis(
                            ap=idx_sb[:, g0 + g : g0 + g + 1], axis=0
                        ),
                    )
                    xe_bf = xgb.tile([P, H], BF16, tag="xeb")
                    nc.vector.tensor_copy(xe_bf[:], xe_sb[:])
                    for k in range(KT_H):
                        tr_ps = ps_tr.tile([P, P], BF16, tag="tr")
                        nc.tensor.transpose(
                            tr_ps[:], xe_bf[:, k * P : (k + 1) * P], ident_bf[:]
                        )
                        nc.vector.tensor_copy(
                            xt_sb[:, k, g * P : (g + 1) * P], tr_ps[:]
                        )
                return xt_sb

            chunks = _chunks(nrows)
            # chunk 0 token loads first so the tensor engine warms up early
            xt_loaded = {0: emit_chunk_load(*chunks[0])}

            b1_sb = cpool.tile([P, KT_F], FP32, tag="b1")
            nc.sync.dma_start(b1_sb[:], b1[:])
            b2_sb = cpool.tile([P, KT_F], FP32, tag="b2")
            nc.sync.dma_start(b2_sb[:], b2[:])
            b3_row = cpool.tile([1, H], FP32, tag="b3row")
            nc.sync.dma_start(b3_row[:], b3[:])
            b3_bc = cpool.tile([P, H], FP32, tag="b3bc")
            nc.gpsimd.partition_broadcast(b3_bc[:], b3_row[0:1, :])

            # resident bf16 weights, staged in consumption order:
            # (w1 c, w2 c) slices for c = 0..3, then w3 (used by the y matmuls)
            w1_bf = wres.tile([P, KT_H, F], BF16, tag="w1")
            w2_bf = wres.tile([P, KT_H, F], BF16, tag="w2")
            w3_bf = wres.tile([P, KT_F, H], BF16, tag="w3")

            def stage(wdram, wbf, k, c):
                st = wstage.tile([P, 512], FP32, tag="wst")
                nc.sync.dma_start(
                    st[:], wdram[k * P : (k + 1) * P, c * 512 : (c + 1) * 512]
                )
                nc.vector.tensor_copy(wbf[:, k, c * 512 : (c + 1) * 512], st[:])

            for c in range(F // 512):
                for k in range(KT_H):
                    stage(w1, w1_bf, k, c)
                for k in range(KT_H):
                    stage(w2, w2_bf, k, c)
            for c in range(H // 512):
                for k in range(KT_F):
                    stage(w3, w3_bf, k, c)

            for ci, (c0, clen) in enumerate(chunks):
                ngr = clen // P
                xt_sb = xt_loaded.pop(ci)

                gt_sb = gtp.tile([P, KT_F, 1024], BF16, tag="gt")
                halves = [(h0, min(512, clen - h0)) for h0 in range(0, clen, 512)]
                for ft in range(KT_F):
                    h1_ps = ps_h1.tile([P, 1024], FP32, tag="h1")
                    h2_ps = ps_h2.tile([P, 1024], FP32, tag="h2")
                    for hps, wbf in ((h1_ps, w1_bf), (h2_ps, w2_bf)):
                        for h0, hlen in halves:
                            for k in range(KT_H):
                                nc.tensor.matmul(
                                    hps[:, h0 : h0 + hlen],
                                    lhsT=wbf[:, k, ft * P : (ft + 1) * P],
                                    rhs=xt_sb[:, k, h0 : h0 + hlen],
                                    start=(k == 0),
                                    stop=(k == KT_H - 1),
                                )
                    if SILU_DECOMPOSE:
                        sg = ew.tile([P, 1024], FP32, tag="sg")
                        nc.scalar.activation(
                            sg[:, :clen], h1_ps[:, :clen], AF.Sigmoid,
                            bias=b1_sb[:, ft : ft + 1], scale=1.0,
                        )
                        h1b = ew.tile([P, 1024], FP32, tag="h1b")
                        nc.scalar.activation(
                            h1b[:, :clen], h1_ps[:, :clen], AF.Identity,
                            bias=b1_sb[:, ft : ft + 1], scale=1.0,
                        )
                        s1 = ew.tile([P, 1024], BF16, tag="s1")
                        nc.vector.tensor_tensor(
                            s1[:, :clen], sg[:, :clen], h1b[:, :clen], op=ALU.mult
                        )
                    else:
                        s1 = ew.tile([P, 1024], BF16, tag="s1")
                        nc.scalar.activation(
                            s1[:, :clen], h1_ps[:, :clen], AF.Silu,
                            bias=b1_sb[:, ft : ft + 1], scale=1.0,
                        )
                    h2b = ew.tile([P, 1024], BF16, tag="h2b")
                    nc.scalar.activation(
                        h2b[:, :clen], h2_ps[:, :clen], AF.Identity,
                        bias=b2_sb[:, ft : ft + 1], scale=1.0,
                    )
                    nc.vector.tensor_tensor(
                        gt_sb[:, ft, :clen], s1[:, :clen], h2b[:, :clen], op=ALU.mult
                    )

                # trace the next chunk's gathers + transposes here so the
                # tensor engine fills the gt-latency bubble before the y phase
                if ci + 1 < len(chunks):
                    xt_loaded[ci + 1] = emit_chunk_load(*chunks[ci + 1])

                for t in range(ngr):
                    y_sb = ysb.tile([P, H], FP32, tag="y")
                    for hc in range(H // 512):
                        y_ps = ps_y.tile([P, 512], FP32, tag="y")
                        for kf in range(KT_F):
                            nc.tensor.matmul(
                                y_ps[:],
                                lhsT=gt_sb[:, kf, t * P : (t + 1) * P],
                                rhs=w3_bf[:, kf, hc * 512 : (hc + 1) * 512],
                                start=(kf == 0),
                                stop=(kf == KT_F - 1),
                            )
                        nc.vector.tensor_tensor(
                            y_sb[:, hc * 512 : (hc + 1) * 512],
                            y_ps[:],
                            b3_bc[:, hc * 512 : (hc + 1) * 512],
                            op=ALU.add,
                        )
                    nc.sync.dma_start(
                        y_out[c0 + t * P : c0 + (t + 1) * P, :], y_sb[:]
                    )
    nc.finalize()
    return nc


_NC_CACHE = {}


def _get_nc(kind, *args):
    key = (kind,) + args
    if key not in _NC_CACHE:
        if kind == "router":
            _NC_CACHE[key] = build_router_nc()
        else:
            _NC_CACHE[key] = build_expert_nc(*args)
    return _NC_CACHE[key]


def _run(nc, in_maps, **kw):
    return run_bass_kernel_spmd(nc, in_maps, core_ids=list(range(N_CORES)), **kw)


def kernel(x, router_w, router_b, w1, b1, w2, b2, w3, b3, _trace=None):
    x = np.ascontiguousarray(np.asarray(x, np.float32))
    router_w = np.ascontiguousarray(np.asarray(router_w, np.float32))
    router_b = np.ascontiguousarray(np.asarray(router_b, np.float32))
    w1 = np.ascontiguousarray(np.asarray(w1, np.float32))
    b1 = np.ascontiguousarray(np.asarray(b1, np.float32))
    w2 = np.ascontiguousarray(np.asarray(w2, np.float32))
    b2 = np.ascontiguousarray(np.asarray(b2, np.float32))
    w3 = np.ascontiguousarray(np.asarray(w3, np.float32))
    b3 = np.ascontiguousarray(np.asarray(b3, np.float32))

    X = x.reshape(S, H)
    # router_w rearranged so SBUF partition p holds rw[k*128+p, e] at [p, k*E+e]
    rw_r = np.ascontiguousarray(
        router_w.reshape(KT_H, P, E).transpose(1, 0, 2).reshape(P, KT_H * E)
    )

    nc1 = _get_nc("router")
    in_maps1 = [
        {
            "xs": X[c * TSHARD : (c + 1) * TSHARD],
            "rw": rw_r,
            "rb": router_b,
        }
        for c in range(N_CORES)
    ]
    res1 = _run(nc1, in_maps1, **({"trace": True, "tmpdir": _trace + "/l1"} if _trace else {}))

    experts = np.empty(S, np.int64)
    gaps = np.empty(S, np.float64)
    ps_total = np.zeros(E, np.float64)
    z2_total = 0.0
    for c in range(N_CORES):
        r = res1.results[c]
        experts[c * TSHARD : (c + 1) * TSHARD] = (
            r["idx_out"].astype(np.int64).T.reshape(TSHARD)
        )
        gaps[c * TSHARD : (c + 1) * TSHARD] = (
            r["gap_out"].astype(np.float64).T.reshape(TSHARD)
        )
        ps_total += r["ps_out"][0].astype(np.float64)
        z2_total += float(r["z2_out"][0, 0])
    l1_ns = res1.exec_time_ns

    # The device router runs in bf16; its top-1 choice is trusted only when the
    # top-2 gap clears a margin that bounds the worst-case bf16 logit error.
    # Near-ties are re-decided from the exact inputs.
    amb = np.nonzero(gaps < 0.06)[0]
    if amb.size:
        lg_fix = X[amb].astype(np.float64) @ router_w.astype(np.float64) + router_b
        experts[amb] = np.argmax(lg_fix, axis=1)

    # capacity truncation (drop_policy='first'), gather lists
    kept_lists = []
    load = np.zeros(E, np.float64)
    for e in range(E):
        toks = np.nonzero(experts == e)[0][:CAP]
        kept_lists.append(toks)
        load[e] = len(toks)
    max_kept = max(len(t) for t in kept_lists)
    nrows = NROWS0 if max_kept <= NROWS0 else CAP
    ng = nrows // P

    nc2 = _get_nc("expert", nrows)
    in_maps2 = []
    for e in range(E):
        toks = kept_lists[e]
        padded = np.zeros(nrows, np.int32)
        padded[: len(toks)] = toks
        idx_arr = np.ascontiguousarray(padded.reshape(ng, P).T)
        b1r = np.ascontiguousarray(b1[e].reshape(KT_F, P).T)
        b2r = np.ascontiguousarray(b2[e].reshape(KT_F, P).T)
        in_maps2.append(
            {
                "xf": X,
                "idx": idx_arr,
                "w1": w1[e],
                "w2": w2[e],
                "w3": w3[e],
                "b1": b1r,
                "b2": b2r,
                "b3": b3[e][None, :],
            }
        )
    res2 = _run(nc2, in_maps2, **({"trace": True, "tmpdir": _trace + "/l2"} if _trace else {}))

    out = np.zeros((S, H), np.float32)
    for e in range(E):
        toks = kept_lists[e]
        out[toks] = res2.results[e]["y_out"][: len(toks)]
    l2_ns = res2.exec_time_ns

    importance = ps_total / float(S)
    aux = float((importance * (load / float(S))).sum() * E * AUX_W)
    aux += z2_total / float(S) * Z_W
    aux = np.float32(aux)

    if _trace:
        kernel.last_exec_ns = (l1_ns, l2_ns)
    return out.reshape(B, T, H), aux


# revision 30
# speedup vs baseline: 1.2245x; 1.0318x over previous
"""Trainium2 Bass kernel for a Switch-style top-1 MoE block (BantamMoEBlock).

Strategy (8 NeuronCores, one full TRN2 chip):
  Launch 1 (router, data-parallel): core c takes token shard c (2048 tokens),
    computes logits = x @ router_w + router_b on the tensor engine (fp32),
    softmax stats + top-1 expert id per token, and the partial sums needed
    for the aux loss (sum of probs per expert, sum of logsumexp^2).
  Host: merges the 8 shards' routing decisions, applies the capacity
    truncation (first-come, cap = 2560), and builds per-expert gather lists.
  Launch 2 (experts, expert-parallel): core e owns expert e. It gathers its
    tokens' rows from a local replica of x via indirect DMA, transposes them
    on the tensor engine, and runs the SwiGLU expert in bf16
    (y = (silu(x@w1+b1) * (x@w2+b2)) @ w3 + b3) with fp32 PSUM accumulation.
  Host: scatters the per-expert outputs back to token order (rows are
    disjoint for top-1 routing) and assembles the scalar aux loss from the
    device partial sums.
"""

import sys

sys.path.insert(0, "/opt/trn_rl_repo")

import numpy as np

import concourse.bass as bass
import concourse.bacc as bacc
import concourse.bass_isa as bass_isa
import concourse.mybir as mybir
import concourse.tile as tile
from concourse.masks import make_identity
from concourse.bass_utils import run_bass_kernel_spmd

P = 128
B, T, H, F, E = 8, 2048, 1024, 2048, 8
S = B * T
CAP = int(np.ceil(1.25 * S / float(E)))  # 2560
NROWS0 = 2304  # 18 groups of 128; covers the worst expert load with slack
AUX_W, Z_W = 0.01, 0.001

FP32 = mybir.dt.float32
BF16 = mybir.dt.bfloat16
I32 = mybir.dt.int32
U32 = mybir.dt.uint32
AF = mybir.ActivationFunctionType
ALU = mybir.AluOpType

N_CORES = 8
TSHARD = S // N_CORES  # 2048 tokens per core in launch 1
KT_H = H // P  # 8 k-tiles over H
KT_F = F // P  # 16 k-tiles over F

# CoreSim does not implement the Silu activation table; set True to build the
# expert kernel with an equivalent sigmoid(x)*x decomposition (sim testing).
SILU_DECOMPOSE = False


def build_router_nc():
    nc = bacc.Bacc("TRN2", target_bir_lowering=False)
    xs = nc.dram_tensor("xs", [TSHARD, H], FP32, kind="ExternalInput")
    rw = nc.dram_tensor("rw", [P, KT_H * E], FP32, kind="ExternalInput")  # [p, k*8+e]
    rb = nc.dram_tensor("rb", [E], FP32, kind="ExternalInput")
    idx_out = nc.dram_tensor("idx_out", [P, TSHARD // P], U32, kind="ExternalOutput")
    gap_out = nc.dram_tensor("gap_out", [P, TSHARD // P], FP32, kind="ExternalOutput")
    ps_out = nc.dram_tensor("ps_out", [1, E], FP32, kind="ExternalOutput")
    z2_out = nc.dram_tensor("z2_out", [1, 1], FP32, kind="ExternalOutput")

    NT = TSHARD // P  # 16 token tiles

    with tile.TileContext(nc) as tc:
        with (
            tc.tile_pool(name="const", bufs=1) as cpool,
            tc.tile_pool(name="work", bufs=3) as wpool,
            tc.tile_pool(name="acc", bufs=1) as apool,
            tc.tile_pool(name="psum", bufs=2, space="PSUM") as pp,
            tc.tile_pool(name="psum_lg", bufs=2, space="PSUM") as plg,
            tc.tile_pool(name="psum_lg2", bufs=2, space="PSUM") as plg2,
        ):
            ident = cpool.tile([P, P], FP32, tag="ident")
            make_identity(nc, ident[:])
            ident_bf = cpool.tile([P, P], BF16, tag="identbf")
            make_identity(nc, ident_bf[:])
            # first token tile's DMA issued before anything else on the queue
            xs0_sb = wpool.tile([P, H], FP32, tag="xs")
            nc.sync.dma_start(xs0_sb[:], xs[0:P, :])
            rw_sb = cpool.tile([P, KT_H, E], FP32, tag="rw")
            nc.sync.dma_start(rw_sb[:], rw[:].rearrange("p (k e) -> p k e", e=E))
            rw_bf = cpool.tile([P, KT_H, E], BF16, tag="rwbf")
            nc.vector.tensor_copy(rw_bf[:], rw_sb[:])
            ones_bf = cpool.tile([1, P], BF16, tag="ones")
            nc.vector.memset(ones_bf[:], 1.0)
            rb_sb = cpool.tile([1, E], FP32, tag="rb")
            nc.sync.dma_start(rb_sb[:], rb[None, :])
            rb_bf = cpool.tile([1, E], BF16, tag="rbbf")
            nc.vector.tensor_copy(rb_bf[:], rb_sb[:])
            iota8 = cpool.tile([P, E], FP32, tag="iota8")
            iota8i = cpool.tile([P, E], I32, tag="iota8i")
            nc.gpsimd.iota(iota8i[:], [[1, E]], channel_multiplier=0)
            nc.vector.tensor_copy(iota8[:], iota8i[:])

            lgall = apool.tile([P, NT, E], FP32, tag="lgall")

            for t in range(NT):
                if t == 0:
                    xs_sb = xs0_sb
                else:
                    xs_sb = wpool.tile([P, H], FP32, tag="xs")
                    nc.sync.dma_start(xs_sb[:], xs[t * P : (t + 1) * P, :])
                xs_bf = wpool.tile([P, H], BF16, tag="xsbf")
                nc.vector.tensor_copy(xs_bf[:], xs_sb[:])
                xt_bf = wpool.tile([P, KT_H, P], BF16, tag="xt")
                for k in range(KT_H):
                    tr_ps = pp.tile([P, P], BF16, tag="tr")
                    nc.tensor.transpose(
                        tr_ps[:], xs_bf[:, k * P : (k + 1) * P], ident_bf[:]
                    )
                    nc.vector.tensor_copy(xt_bf[:, k, :], tr_ps[:])
                # logits.T [E, P] with router weights as the stationary operand
                lgT_ps = plg.tile([E, P], FP32, tag="lgT")
                for k in range(KT_H):
                    nc.tensor.matmul(
                        lgT_ps[:],
                        lhsT=rw_bf[:, k, :],
                        rhs=xt_bf[:, k, :],
                        start=(k == 0),
                        stop=False,
                    )
                # += rb.T @ ones : adds router_b[e] to every token's logits
                nc.tensor.matmul(
                    lgT_ps[:],
                    lhsT=rb_bf[0:1, :],
                    rhs=ones_bf[0:1, :],
                    start=False,
                    stop=True,
                )
                lgT_sb = wpool.tile([E, P], FP32, tag="lgT_sb")
                nc.vector.tensor_copy(lgT_sb[:], lgT_ps[:])
                lg_ps2 = plg2.tile([P, E], FP32, tag="lg2")
                nc.tensor.transpose(lg_ps2[:], lgT_sb[:], ident[0:E, 0:E])
                nc.vector.tensor_copy(lgall[:, t, :], lg_ps2[:])

            # batched softmax / logsumexp / argmax stats over all NT tiles
            m16 = apool.tile([P, NT], FP32, tag="m16")
            nc.vector.tensor_reduce(
                m16[:], lgall[:], axis=mybir.AxisListType.X, op=ALU.max
            )
            m16b = m16[:, :, None].to_broadcast([P, NT, E])
            lgc = apool.tile([P, NT, E], FP32, tag="lgc")
            nc.vector.tensor_tensor(lgc[:], lgall[:], m16b, op=ALU.subtract)
            pex = apool.tile([P, NT, E], FP32, tag="pex")
            nc.scalar.activation(
                pex[:].rearrange("p t e -> p (t e)"),
                lgc[:].rearrange("p t e -> p (t e)"),
                AF.Exp,
            )
            se16 = apool.tile([P, NT], FP32, tag="se16")
            nc.vector.tensor_reduce(
                se16[:], pex[:], axis=mybir.AxisListType.X, op=ALU.add
            )
            rse16 = apool.tile([P, NT], FP32, tag="rse16")
            nc.vector.reciprocal(rse16[:], se16[:])
            pn = apool.tile([P, NT, E], FP32, tag="pn")
            nc.vector.tensor_tensor(
                pn[:], pex[:], rse16[:, :, None].to_broadcast([P, NT, E]), op=ALU.mult
            )
            psum8 = apool.tile([P, E], FP32, tag="psum8")
            nc.vector.tensor_reduce(
                psum8[:], pn[:].rearrange("p t e -> p e t"),
                axis=mybir.AxisListType.X, op=ALU.add,
            )
            # argmax per (token): sum(e * (lg == max)); top-2 gap for the host
            eq = apool.tile([P, NT, E], FP32, tag="eq")
            nc.vector.tensor_tensor(eq[:], lgall[:], m16b, op=ALU.is_equal)
            lg2nd = apool.tile([P, NT, E], FP32, tag="lg2nd")
            nc.vector.tensor_scalar_mul(lg2nd[:], eq[:], -1e30)
            nc.vector.tensor_tensor(lg2nd[:], lg2nd[:], lgall[:], op=ALU.add)
            m2 = apool.tile([P, NT], FP32, tag="m2")
            nc.vector.tensor_reduce(
                m2[:], lg2nd[:], axis=mybir.AxisListType.X, op=ALU.max
            )
            gap16 = apool.tile([P, NT], FP32, tag="gap16")
            nc.vector.tensor_tensor(gap16[:], m16[:], m2[:], op=ALU.subtract)
            nc.vector.tensor_tensor(
                eq[:], eq[:], iota8[:, None, :].to_broadcast([P, NT, E]), op=ALU.mult
            )
            idxf = apool.tile([P, NT], FP32, tag="idxf")
            nc.vector.tensor_reduce(
                idxf[:], eq[:], axis=mybir.AxisListType.X, op=ALU.add
            )
            idxall = apool.tile([P, NT], U32, tag="idxall")
            nc.vector.tensor_copy(idxall[:], idxf[:])

            lnse = apool.tile([P, NT], FP32, tag="lnse")
            nc.scalar.activation(lnse[:], se16[:], AF.Ln)
            zall = apool.tile([P, NT], FP32, tag="zall")
            nc.vector.tensor_tensor(zall[:], lnse[:], m16[:], op=ALU.add)
            z2p = apool.tile([P, 1], FP32, tag="z2p")
            z2sq = apool.tile([P, NT], FP32, tag="z2sq")
            nc.scalar.activation(z2sq[:], zall[:], AF.Square, accum_out=z2p[:])

            z2r = apool.tile([P, 1], FP32, tag="z2r")
            nc.gpsimd.partition_all_reduce(
                z2r[:], z2p[:], channels=P, reduce_op=bass_isa.ReduceOp.add
            )
            pr = apool.tile([P, E], FP32, tag="pr")
            nc.gpsimd.partition_all_reduce(
                pr[:], psum8[:], channels=P, reduce_op=bass_isa.ReduceOp.add
            )
            nc.sync.dma_start(idx_out[:], idxall[:])
            nc.sync.dma_start(gap_out[:], gap16[:])
            nc.sync.dma_start(ps_out[:], pr[0:1, :])
            nc.sync.dma_start(z2_out[:], z2r[0:1, :])
    nc.finalize()
    return nc


def _chunks(nrows):
    out = []
    pos = 0
    while pos < nrows:
        c = min(1024, nrows - pos)
        out.append((pos, c))
        pos += c
    return out


def build_expert_nc(nrows):
    assert nrows % P == 0
    ng = nrows // P
    nc = bacc.Bacc("TRN2", target_bir_lowering=False)
    xf = nc.dram_tensor("xf", [S, H], FP32, kind="ExternalInput")
    idx = nc.dram_tensor("idx", [P, ng], I32, kind="ExternalInput")
    w1 = nc.dram_tensor("w1", [H, F], FP32, kind="ExternalInput")
    w2 = nc.dram_tensor("w2", [H, F], FP32, kind="ExternalInput")
    w3 = nc.dram_tensor("w3", [F, H], FP32, kind="ExternalInput")
    b1 = nc.dram_tensor("b1", [P, KT_F], FP32, kind="ExternalInput")  # b1[f] at [f%128, f//128]
    b2 = nc.dram_tensor("b2", [P, KT_F], FP32, kind="ExternalInput")
    b3 = nc.dram_tensor("b3", [1, H], FP32, kind="ExternalInput")
    y_out = nc.dram_tensor("y_out", [nrows, H], FP32, kind="ExternalOutput")

    with tile.TileContext(nc) as tc:
        with (
            tc.tile_pool(name="const", bufs=1) as cpool,
            tc.tile_pool(name="wres", bufs=1) as wres,
            tc.tile_pool(name="wstage", bufs=2) as wstage,  # [P, 2048] f32 slots
            tc.tile_pool(name="xg", bufs=2) as xg,
            tc.tile_pool(name="xgb", bufs=2) as xgb,
            tc.tile_pool(name="xt", bufs=2) as xtp,
            tc.tile_pool(name="gt", bufs=1) as gtp,
            tc.tile_pool(name="ysb", bufs=1) as ysb,
            tc.tile_pool(name="ew", bufs=2) as ew,
            tc.tile_pool(name="ps_tr", bufs=2, space="PSUM") as ps_tr,
            tc.tile_pool(name="ps_h1", bufs=1, space="PSUM") as ps_h1,
            tc.tile_pool(name="ps_h2", bufs=1, space="PSUM") as ps_h2,
            tc.tile_pool(name="ps_y", bufs=2, space="PSUM") as ps_y,
        ):
            ident_bf = cpool.tile([P, P], BF16, tag="identbf")
            make_identity(nc, ident_bf[:])
            idx_sb = cpool.tile([P, ng], I32, tag="idx")
            nc.sync.dma_start(idx_sb[:], idx[:])

            def emit_chunk_load(c0, clen):
                ngr = clen // P
                g0 = c0 // P
                xt_sb = xtp.tile([P, KT_H, 1024], BF16, tag="xt")
                for g in range(ngr):
                    xe_sb = xg.tile([P, H], FP32, tag="xe")
                    nc.gpsimd.indirect_dma_start(
                        out=xe_sb[:],
                        out_offset=None,
                        in_=xf[:],
                        in_offset=bass.IndirectOffsetOnAxis(
                            ap=idx_sb[:, g0 + g : g0 + g + 1], axis=0
                        ),
                    )
                    xe_bf = xgb.tile([P, H], BF16, tag="xeb")
                    nc.vector.tensor_copy(xe_bf[:], xe_sb[:])
                    for k in range(KT_H):
                        tr_ps = ps_tr.tile([P, P], BF16, tag="tr")
                        nc.tensor.transpose(
                            tr_ps[:], xe_bf[:, k * P : (k + 1) * P], ident_bf[:]
                        )
                        # ACT (Identity) does the PSUM->SBUF copy; it shares the
                        # already-loaded activation table with Silu
                        nc.scalar.activation(
                            xt_sb[:, k, g * P : (g + 1) * P], tr_ps[:], AF.Identity
                        )
                return xt_sb

            # resident bf16 weights; the f32 source is staged as whole k-tiles
            # ([128, 2048] ~ 1 MB DMAs) to keep the DMA->cast chain short
            w1_bf = wres.tile([P, KT_H, F], BF16, tag="w1")
            w2_bf = wres.tile([P, KT_H, F], BF16, tag="w2")
            w3_bf = wres.tile([P, KT_F, H], BF16, tag="w3")

            def stage_ktile(src_ap, wbf_slice):
                st = wstage.tile([P, 2048], FP32, tag="wst")
                nc.sync.dma_start(st[:], src_ap)
                nc.vector.tensor_copy(wbf_slice, st[:])

            def emit_weight_dmas():
                for k in range(KT_H):
                    stage_ktile(w1[k * P : (k + 1) * P, :], w1_bf[:, k, :])
                for k in range(KT_H):
                    stage_ktile(w2[k * P : (k + 1) * P, :], w2_bf[:, k, :])
                for k2 in range(KT_F // 2):
                    st = wstage.tile([P, 2048], FP32, tag="wst")
                    nc.sync.dma_start(
                        st[:].rearrange("p (a h) -> p a h", a=2),
                        w3[k2 * 2 * P : (k2 + 1) * 2 * P, :].rearrange(
                            "(a p) h -> p a h", p=P
                        ),
                    )
                    nc.vector.tensor_copy(w3_bf[:, k2 * 2 : k2 * 2 + 2, :], st[:])

            chunks = _chunks(nrows)
            # chunk 0 token loads first so the tensor engine warms up early
            xt_loaded = {0: emit_chunk_load(*chunks[0])}
            emit_weight_dmas()

            b1_sb = cpool.tile([P, KT_F], FP32, tag="b1")
            nc.sync.dma_start(b1_sb[:], b1[:])
            b2_sb = cpool.tile([P, KT_F], FP32, tag="b2")
            nc.sync.dma_start(b2_sb[:], b2[:])
            b3_bc = cpool.tile([P, H], FP32, tag="b3bc")
            nc.sync.dma_start(b3_bc[:], b3[0:1, :].to_broadcast([P, H]))

            for ci, (c0, clen) in enumerate(chunks):
                ngr = clen // P
                xt_sb = xt_loaded.pop(ci)

                gt_sb = gtp.tile([P, KT_F, 1024], BF16, tag="gt")
                halves = [(h0, min(512, clen - h0)) for h0 in range(0, clen, 512)]
                for ft in range(KT_F):
                    h1_ps = ps_h1.tile([P, 1024], FP32, tag="h1")
                    h2_ps = ps_h2.tile([P, 1024], FP32, tag="h2")
                    for hps, wbf in ((h1_ps, w1_bf), (h2_ps, w2_bf)):
                        for h0, hlen in halves:
                            for k in range(KT_H):
                                nc.tensor.matmul(
                                    hps[:, h0 : h0 + hlen],
                                    lhsT=wbf[:, k, ft * P : (ft + 1) * P],
                                    rhs=xt_sb[:, k, h0 : h0 + hlen],
                                    start=(k == 0),
                                    stop=(k == KT_H - 1),
                                )
                    if SILU_DECOMPOSE:
                        sg = ew.tile([P, 1024], BF16, tag="s1")
                        nc.scalar.activation(
                            sg[:, :clen], h1_ps[:, :clen], AF.Sigmoid,
                            bias=b1_sb[:, ft : ft + 1], scale=1.0,
                        )
                        h1b = ew.tile([P, 1024], BF16, tag="h2b")
                        nc.scalar.activation(
                            h1b[:, :clen], h1_ps[:, :clen], AF.Identity,
                            bias=b1_sb[:, ft : ft + 1], scale=1.0,
                        )
                        s1 = ew.tile([P, 1024], BF16, tag="s1")
                        nc.vector.tensor_tensor(
                            s1[:, :clen], sg[:, :clen], h1b[:, :clen], op=ALU.mult
                        )
                    else:
                        s1 = ew.tile([P, 1024], BF16, tag="s1")
                        nc.scalar.activation(
                            s1[:, :clen], h1_ps[:, :clen], AF.Silu,
                            bias=b1_sb[:, ft : ft + 1], scale=1.0,
                        )
                    h2b = ew.tile([P, 1024], BF16, tag="h2b")
                    nc.scalar.activation(
                        h2b[:, :clen], h2_ps[:, :clen], AF.Identity,
                        bias=b2_sb[:, ft : ft + 1], scale=1.0,
                    )
                    nc.vector.tensor_tensor(
                        gt_sb[:, ft, :clen], s1[:, :clen], h2b[:, :clen], op=ALU.mult
                    )

                # trace the next chunk's gathers + transposes here so the
                # tensor engine fills the gt-latency bubble before the y phase
                if ci + 1 < len(chunks):
                    xt_loaded[ci + 1] = emit_chunk_load(*chunks[ci + 1])

                for t in range(ngr):
                    y_sb = ysb.tile([P, H], FP32, tag="y")
                    for hc in range(H // 512):
                        y_ps = ps_y.tile([P, 512], FP32, tag="y")
                        for kf in range(KT_F):
                            nc.tensor.matmul(
                                y_ps[:],
                                lhsT=gt_sb[:, kf, t * P : (t + 1) * P],
                                rhs=w3_bf[:, kf, hc * 512 : (hc + 1) * 512],
                                start=(kf == 0),
                                stop=(kf == KT_F - 1),
                            )
                        nc.vector.tensor_tensor(
                            y_sb[:, hc * 512 : (hc + 1) * 512],
                            y_ps[:],
                            b3_bc[:, hc * 512 : (hc + 1) * 512],
                            op=ALU.add,
                        )
                    nc.sync.dma_start(
                        y_out[c0 + t * P : c0 + (t + 1) * P, :], y_sb[:]
                    )
    nc.finalize()
    return nc


_NC_CACHE = {}


def _get_nc(kind, *args):
    key = (kind,) + args
    if key not in _NC_CACHE:
        if kind == "router":
            _NC_CACHE[key] = build_router_nc()
        else:
            _NC_CACHE[key] = build_expert_nc(*args)
    return _NC_CACHE[key]


def _run(nc, in_maps, **kw):
    return run_bass_kernel_spmd(nc, in_maps, core_ids=list(range(N_CORES)), **kw)


def kernel(x, router_w, router_b, w1, b1, w2, b2, w3, b3, _trace=None):
    x = np.ascontiguousarray(np.asarray(x, np.float32))
    router_w = np.ascontiguousarray(np.asarray(router_w, np.float32))
    router_b = np.ascontiguousarray(np.asarray(router_b, np.float32))
    w1 = np.ascontiguousarray(np.asarray(w1, np.float32))
    b1 = np.ascontiguousarray(np.asarray(b1, np.float32))
    w2 = np.ascontiguousarray(np.asarray(w2, np.float32))
    b2 = np.ascontiguousarray(np.asarray(b2, np.float32))
    w3 = np.ascontiguousarray(np.asarray(w3, np.float32))
    b3 = np.ascontiguousarray(np.asarray(b3, np.float32))

    X = x.reshape(S, H)
    # router_w rearranged so SBUF partition p holds rw[k*128+p, e] at [p, k*E+e]
    rw_r = np.ascontiguousarray(
        router_w.reshape(KT_H, P, E).transpose(1, 0, 2).reshape(P, KT_H * E)
    )

    nc1 = _get_nc("router")
    in_maps1 = [
        {
            "xs": X[c * TSHARD : (c + 1) * TSHARD],
            "rw": rw_r,
            "rb": router_b,
        }
        for c in range(N_CORES)
    ]
    res1 = _run(nc1, in_maps1, **({"trace": True, "tmpdir": _trace + "/l1"} if _trace else {}))

    experts = np.empty(S, np.int64)
    gaps = np.empty(S, np.float64)
    ps_total = np.zeros(E, np.float64)
    z2_total = 0.0
    for c in range(N_CORES):
        r = res1.results[c]
        experts[c * TSHARD : (c + 1) * TSHARD] = (
            r["idx_out"].astype(np.int64).T.reshape(TSHARD)
        )
        gaps[c * TSHARD : (c + 1) * TSHARD] = (
            r["gap_out"].astype(np.float64).T.reshape(TSHARD)
        )
        ps_total += r["ps_out"][0].astype(np.float64)
        z2_total += float(r["z2_out"][0, 0])
    l1_ns = res1.exec_time_ns

    # The device router runs in bf16; its top-1 choice is trusted only when the
    # top-2 gap clears a margin that bounds the worst-case bf16 logit error.
    # Near-ties are re-decided from the exact inputs.
    amb = np.nonzero(gaps < 0.06)[0]
    if amb.size:
        lg_fix = X[amb].astype(np.float64) @ router_w.astype(np.float64) + router_b
        experts[amb] = np.argmax(lg_fix, axis=1)

    # capacity truncation (drop_policy='first'), gather lists
    kept_lists = []
    load = np.zeros(E, np.float64)
    for e in range(E):
        toks = np.nonzero(experts == e)[0][:CAP]
        kept_lists.append(toks)
        load[e] = len(toks)
    max_kept = max(len(t) for t in kept_lists)
    nrows = NROWS0 if max_kept <= NROWS0 else CAP
    ng = nrows // P

    nc2 = _get_nc("expert", nrows)
    in_maps2 = []
    for e in range(E):
        toks = kept_lists[e]
        padded = np.zeros(nrows, np.int32)
        padded[: len(toks)] = toks
        idx_arr = np.ascontiguousarray(padded.reshape(ng, P).T)
        b1r = np.ascontiguousarray(b1[e].reshape(KT_F, P).T)
        b2r = np.ascontiguousarray(b2[e].reshape(KT_F, P).T)
        in_maps2.append(
            {
                "xf": X,
                "idx": idx_arr,
                "w1": w1[e],
                "w2": w2[e],
                "w3": w3[e],
                "b1": b1r,
                "b2": b2r,
                "b3": b3[e][None, :],
            }
        )
    res2 = _run(nc2, in_maps2, **({"trace": True, "tmpdir": _trace + "/l2"} if _trace else {}))

    out = np.zeros((S, H), np.float32)
    for e in range(E):
        toks = kept_lists[e]
        out[toks] = res2.results[e]["y_out"][: len(toks)]
    l2_ns = res2.exec_time_ns

    importance = ps_total / float(S)
    aux = float((importance * (load / float(S))).sum() * E * AUX_W)
    aux += z2_total / float(S) * Z_W
    aux = np.float32(aux)

    if _trace:
        kernel.last_exec_ns = (l1_ns, l2_ns)
    return out.reshape(B, T, H), aux
